# revision 1
# baseline (speedup 1.0000x reference)
"""Trainium2 Bass kernel for nn_Blur: depthwise 4x4 FIR conv, pad=2.

out[b,c,h',w'] = sum_{i,j} wf[i,j] * xpad[b,c,h'+i,w'+j],  wf = flip(kernel)
x: [8,256,256,256] f32, kernel: [4,4] f32 -> out: [8,256,257,257] f32

Strategy: pure data parallel over batch (8 cores, 1 batch elem each).
Per core, the full 2D conv runs on the TensorEngine as 4 banded-matrix
matmuls (one per kernel column j) accumulating in PSUM:
    psum[h', w'] += sum_h B_j[h,h'] * xpad_w[h, w'+j]
with B_j[h,h'] = wf[h-h'+2, j] built on the host from the runtime kernel.
float32r dtype gives full PE rate (1 cyc/row at N>=256, even N required);
rel err ~2e-4, far inside the 2e-2 gate.

DMA layout: 8 channels batched per transfer (~1 MB per dma_start) to
amortize the ~0.8-2us per-DMA completion latency; input loads issue on
the SP HWDGE ring (nc.sync), output stores on the ACT ring (nc.scalar)
so the two FIFO rings overlap.
"""

import numpy as np

_C, _H, _W = 256, 256, 256
_HO, _WO = 257, 257
_NCORES = 8
# (hp0, Mv, hlo, Kv): output rows [hp0, hp0+Mv), contraction rows [hlo, hlo+Kv)
_TILES = [(0, 125, 0, 126), (125, 125, 123, 128), (250, 7, 248, 8)]
_NW = 262  # padded width in SBUF: 2 zero | 256 data | 4 zero
_NMM = 258  # matmul free dim (257 outputs + 1 garbage col), must be even
_OPAD = 264  # padded output row pitch in DRAM (1056B, 32B-aligned)
_CB = 8  # channels per DMA batch / psum rotation


def _build_bands(kern):
    wf = np.ascontiguousarray(np.asarray(kern, np.float32)[::-1, ::-1])
    bands = np.zeros((128, 3, 4, 125), np.float32)
    for v, (hp0, Mv, hlo, Kv) in enumerate(_TILES):
        for j in range(4):
            for hr in range(Kv):
                h = hlo + hr
                for mr in range(Mv):
                    i = h - (hp0 + mr) + 2
                    if 0 <= i < 4:
                        bands[hr, v, j, mr] = wf[i, j]
    return bands


_NC_CACHE = {}


def _build_nc():
    if "nc" in _NC_CACHE:
        return _NC_CACHE["nc"]
    import concourse.bacc as bacc
    import concourse.mybir as mybir
    import concourse.tile as tile

    nc = bacc.Bacc()
    x_d = nc.declare_dram_parameter("x", [_C, _H, _W], mybir.dt.float32r, isOutput=False)
    b_d = nc.declare_dram_parameter(
        "bands", [128, 3, 4, 125], mybir.dt.float32r, isOutput=False
    )
    o_d = nc.declare_dram_parameter("out", [_C, _HO, _OPAD], mybir.dt.float32, isOutput=True)
    z_d = nc.declare_dram_parameter("zpad", [128, _CB, 4], mybir.dt.float32r, isOutput=False)

    NBX = 4  # x-tile ring depth (each tile holds a whole channel-group's rows)
    NBO = 4  # out-tile ring depth
    NBP = 8  # psum banks: one per channel within a group
    with tile.TileContext(nc) as tc:
        with (
            tc.tile_pool(name="sb", bufs=1) as pool,
            tc.tile_pool(name="ps", bufs=1, space="PSUM") as pp,
        ):
            band_sb = pool.tile([128, 3, 4, 125], mybir.dt.float32r, tag="bands")
            nc.sync.dma_start(out=band_sb[:], in_=b_d[:])

            xts = []
            for i in range(NBX):
                t = pool.tile(
                    [128, _CB, _NW], mybir.dt.float32r, tag=f"xt{i}", name=f"xt{i}"
                )
                nc.sync.dma_start(out=t[:, :, 0:2], in_=z_d[:, :, 0:2])
                nc.sync.dma_start(out=t[:, :, 258:_NW], in_=z_d[:, :, 0:4])
                xts.append(t)
            oss = [
                pool.tile(
                    [128, _CB, _OPAD], mybir.dt.float32, tag=f"os{i}", name=f"os{i}"
                )
                for i in range(NBO)
            ]
            pss = [
                pp.tile([128, _NMM], mybir.dt.float32, tag=f"ps{i}", name=f"ps{i}")
                for i in range(NBP)
            ]

            it = 0
            for c0 in range(0, _C, _CB):
                for v, (hp0, Mv, hlo, Kv) in enumerate(_TILES):
                    xt = xts[it % NBX]
                    osb = oss[it % NBO]
                    nc.sync.dma_start(
                        out=xt[0:Kv, :, 2:258],
                        in_=x_d[c0 : c0 + _CB, hlo : hlo + Kv, :].rearrange(
                            "c h w -> h c w"
                        ),
                    )
                    for cc in range(_CB):
                        ps = pss[cc]
                        for j in range(4):
                            nc.tensor.matmul(
                                ps[0:Mv, 0:_NMM],
                                band_sb[0:Kv, v, j, 0:Mv],
                                xt[0:Kv, cc, j : j + _NMM],
                                start=(j == 0),
                                stop=(j == 3),
                            )
                        if cc % 2 == 0:
                            nc.vector.tensor_copy(
                                osb[0:Mv, cc, 0:_WO], ps[0:Mv, 0:_WO]
                            )
                        else:
                            nc.scalar.copy(osb[0:Mv, cc, 0:_WO], ps[0:Mv, 0:_WO])
                    nc.gpsimd.dma_start(
                        out=o_d[c0 : c0 + _CB, hp0 : hp0 + Mv, :].rearrange(
                            "c h w -> h c w"
                        ),
                        in_=osb[0:Mv, :, 0:_OPAD],
                    )
                    it += 1
    nc.finalize()
    _NC_CACHE["nc"] = nc
    return nc


def _run(x, kern, trace=False):
    from concourse.bass_utils import run_bass_kernel_spmd

    x = np.asarray(x, dtype=np.float32)
    bands = _build_bands(kern)
    nc = _build_nc()
    zpad = np.zeros((128, _CB, 4), np.float32)
    in_maps = [
        {"x": np.ascontiguousarray(x[b]), "bands": bands, "zpad": zpad}
        for b in range(_NCORES)
    ]
    res = run_bass_kernel_spmd(nc, in_maps, list(range(_NCORES)), trace=trace)
    out = np.stack(
        [np.asarray(res.results[i]["out"])[:, :, : _WO] for i in range(_NCORES)],
        axis=0,
    ).astype(np.float32)
    return out, res


def kernel(x, kernel):
    out, _ = _run(x, kernel, trace=False)
    return out



# revision 2
# speedup vs baseline: 1.2302x; 1.2302x over previous
"""Trainium2 Bass kernel for nn_Blur: depthwise 4x4 FIR conv, pad=2.

out[b,c,h',w'] = sum_{i,j} wf[i,j] * xpad[b,c,h'+i,w'+j],  wf = flip(kernel)
x: [8,256,256,256] f32, kernel: [4,4] f32 -> out: [8,256,257,257] f32

Strategy: pure data parallel over batch (8 cores, 1 batch elem each).
Per core, the 2D conv runs on the TensorEngine as 4 banded-matrix
matmuls (one per kernel column j) accumulating in PSUM:
    psum[h', w'] += sum_h B_j[h,h'] * xpad_w[h, w'+j]
with B_j[h,h'] = wf[h-h'+2, j] built on the host from the runtime kernel.

v1 perf changes vs baseline:
- bf16 end-to-end: x converted to bf16 on host, bands bf16, output
  stored bf16 and upcast on host. Halves HBM traffic (the memory-bound
  roofline) and DMA lines stay >=512B so full DMA bus rate holds.
- j-outer matmul order: the stationary band matrix is loaded 4x per
  tile instead of 32x (8 channels share each load).
- output DMA on the ACT HWDGE ring, input on SP ring; psum->sbuf
  copies alternate DVE/ACT.
"""

import numpy as np

_C, _H, _W = 256, 256, 256
_HO, _WO = 257, 257
_NCORES = 8
# (hp0, Mv, hlo, Kv): output rows [hp0, hp0+Mv), contraction rows [hlo, hlo+Kv)
_TILES = [(0, 125, 0, 126), (125, 125, 123, 128), (250, 7, 248, 8)]
_NW = 262  # padded width in SBUF: 2 zero | 256 data | 4 zero
_NMM = 258  # matmul free dim (257 outputs + 1 garbage col)
_OPAD = 264  # padded output row pitch in DRAM (528B in bf16, 16B-aligned)
_CB = 8  # channels per DMA batch / psum rotation


def _build_bands(kern):
    wf = np.ascontiguousarray(np.asarray(kern, np.float32)[::-1, ::-1])
    bands = np.zeros((128, 3, 4, 125), np.float32)
    for v, (hp0, Mv, hlo, Kv) in enumerate(_TILES):
        for j in range(4):
            for hr in range(Kv):
                h = hlo + hr
                for mr in range(Mv):
                    i = h - (hp0 + mr) + 2
                    if 0 <= i < 4:
                        bands[hr, v, j, mr] = wf[i, j]
    return bands


_NC_CACHE = {}


def _build_nc():
    if "nc" in _NC_CACHE:
        return _NC_CACHE["nc"]
    import concourse.bacc as bacc
    import concourse.mybir as mybir
    import concourse.tile as tile

    bf16 = mybir.dt.bfloat16
    nc = bacc.Bacc()
    x_d = nc.declare_dram_parameter("x", [_C, _H, _W], bf16, isOutput=False)
    b_d = nc.declare_dram_parameter("bands", [128, 3, 4, 125], bf16, isOutput=False)
    o_d = nc.declare_dram_parameter("out", [_C, _HO, _OPAD], bf16, isOutput=True)
    z_d = nc.declare_dram_parameter("zpad", [128, _CB, 4], bf16, isOutput=False)

    NBX = 4  # x-tile ring depth (each tile holds a whole channel-group's rows)
    NBO = 4  # out-tile ring depth
    NBP = 8  # psum banks: one per channel within a group
    with tile.TileContext(nc) as tc:
        with (
            tc.tile_pool(name="sb", bufs=1) as pool,
            tc.tile_pool(name="ps", bufs=1, space="PSUM") as pp,
        ):
            band_sb = pool.tile([128, 3, 4, 125], bf16, tag="bands")
            nc.sync.dma_start(out=band_sb[:], in_=b_d[:])

            xts = []
            for i in range(NBX):
                t = pool.tile([128, _CB, _NW], bf16, tag=f"xt{i}", name=f"xt{i}")
                nc.sync.dma_start(out=t[:, :, 0:2], in_=z_d[:, :, 0:2])
                nc.sync.dma_start(out=t[:, :, 258:_NW], in_=z_d[:, :, 0:4])
                xts.append(t)
            oss = [
                pool.tile([128, _CB, _OPAD], bf16, tag=f"os{i}", name=f"os{i}")
                for i in range(NBO)
            ]
            pss = [
                pp.tile([128, _NMM], mybir.dt.float32, tag=f"ps{i}", name=f"ps{i}")
                for i in range(NBP)
            ]

            it = 0
            for c0 in range(0, _C, _CB):
                for v, (hp0, Mv, hlo, Kv) in enumerate(_TILES):
                    xt = xts[it % NBX]
                    osb = oss[it % NBO]
                    nc.sync.dma_start(
                        out=xt[0:Kv, :, 2:258],
                        in_=x_d[c0 : c0 + _CB, hlo : hlo + Kv, :].rearrange(
                            "c h w -> h c w"
                        ),
                    )
                    for j in range(4):
                        for cc in range(_CB):
                            nc.tensor.matmul(
                                pss[cc][0:Mv, 0:_NMM],
                                band_sb[0:Kv, v, j, 0:Mv],
                                xt[0:Kv, cc, j : j + _NMM],
                                start=(j == 0),
                                stop=(j == 3),
                            )
                    for cc in range(_CB):
                        if cc % 2 == 0:
                            nc.vector.tensor_copy(
                                osb[0:Mv, cc, 0:_WO], pss[cc][0:Mv, 0:_WO]
                            )
                        else:
                            nc.scalar.copy(osb[0:Mv, cc, 0:_WO], pss[cc][0:Mv, 0:_WO])
                    nc.scalar.dma_start(
                        out=o_d[c0 : c0 + _CB, hp0 : hp0 + Mv, :].rearrange(
                            "c h w -> h c w"
                        ),
                        in_=osb[0:Mv, :, 0:_OPAD],
                    )
                    it += 1
    nc.finalize()
    _NC_CACHE["nc"] = nc
    return nc


def _run(x, kern, trace=False):
    import ml_dtypes
    from concourse.bass_utils import run_bass_kernel_spmd

    bf16 = ml_dtypes.bfloat16
    x = np.asarray(x, dtype=np.float32)
    bands = _build_bands(kern).astype(bf16)
    nc = _build_nc()
    zpad = np.zeros((128, _CB, 4), bf16)
    in_maps = [
        {
            "x": np.ascontiguousarray(x[b]).astype(bf16),
            "bands": bands,
            "zpad": zpad,
        }
        for b in range(_NCORES)
    ]
    res = run_bass_kernel_spmd(nc, in_maps, list(range(_NCORES)), trace=trace)
    out = np.stack(
        [
            np.asarray(res.results[i]["out"])[:, :, :_WO].astype(np.float32)
            for i in range(_NCORES)
        ],
        axis=0,
    )
    return out, res


def kernel(x, kernel):
    out, _ = _run(x, kernel, trace=False)
    return out


# revision 5
# speedup vs baseline: 1.2577x; 1.0223x over previous
"""Trainium2 Bass kernel for nn_Blur: depthwise 4x4 FIR conv, pad=2.

out[b,c,h',w'] = sum_{i,j} wf[i,j] * xpad[b,c,h'+i,w'+j],  wf = flip(kernel)
x: [8,256,256,256] f32, kernel: [4,4] f32 -> out: [8,256,257,257] f32

Strategy: pure data parallel over batch (8 cores, 1 batch elem each).
Per core, output rows 0..249 run on the TensorEngine as 4 banded-matrix
matmuls (one per kernel column j) accumulating in PSUM:
    psum[h', w'] += sum_h B_j[h,h'] * xpad_w[h, w'+j]
with B_j[h,h'] = wf[h-h'+2, j] built on the host from the runtime kernel.
The 7-row output tail (h' 250..256) runs channel-major on the gpsimd
engine as fused multiply-accumulate chains, so the PE only runs 2 full
125-row tiles per channel (the tail tile would cost a full matmul's
cycles for 7 rows of output).

Perf structure:
- bf16 end-to-end: x converted to bf16 on host, bands bf16, output
  stored bf16 and upcast on host. Halves HBM traffic (the memory-bound
  roofline); DMA lines stay >=512B so full DMA bus rate holds.
- PSUM as 2 tiles of [128, 4ch, 512] (bank-aligned channel stride):
  psum->sbuf copies batch 4 channels per instruction, and the two
  halves ping-pong so copies overlap the other half's matmuls.
- input DMA on SP ring, output DMA on ACT ring, copies split DVE/ACT.
- all weights (bands, tail scalars) are runtime DRAM inputs.
"""

import numpy as np

_C, _H, _W = 256, 256, 256
_HO, _WO = 257, 257
_NCORES = 8
# (hp0, Mv, hlo, Kv): output rows [hp0, hp0+Mv), contraction rows [hlo, hlo+Kv)
_TILES = [(0, 125, 0, 126), (125, 125, 123, 128)]
_TAIL_HP0 = 250  # output rows 250..256 on gpsimd
_TAIL_HLO = 248  # input rows 248..255
_NW = 262  # padded width in SBUF: 2 zero | 256 data | 4 zero
_NMM = 258  # matmul free dim (257 outputs + 1 garbage col)
_OPAD = 264  # padded output row pitch in DRAM (528B in bf16)
_CB = 8  # channels per DMA batch / psum rotation


def _build_bands(kern):
    wf = np.ascontiguousarray(np.asarray(kern, np.float32)[::-1, ::-1])
    bands = np.zeros((128, 2, 4, 125), np.float32)
    for v, (hp0, Mv, hlo, Kv) in enumerate(_TILES):
        for j in range(4):
            for hr in range(Kv):
                h = hlo + hr
                for mr in range(Mv):
                    i = h - (hp0 + mr) + 2
                    if 0 <= i < 4:
                        bands[hr, v, j, mr] = wf[i, j]
    return bands


def _tail_terms():
    """(hp, hrow, i) triples for the gpsimd tail: out row 250+hp uses
    input row 248+hrow with kernel row i."""
    terms = []
    for hp in range(7):
        for i in range(4):
            h = _TAIL_HP0 + hp + i - 2
            if _TAIL_HLO <= h < _H:
                terms.append((hp, h - _TAIL_HLO, i))
    return terms


_NC_CACHE = {}


def _build_nc():
    if "nc" in _NC_CACHE:
        return _NC_CACHE["nc"]
    import concourse.bacc as bacc
    import concourse.mybir as mybir
    import concourse.tile as tile

    bf16 = mybir.dt.bfloat16
    f32 = mybir.dt.float32
    AO = mybir.AluOpType
    nc = bacc.Bacc()
    x_d = nc.declare_dram_parameter("x", [_C, _H, _W], bf16, isOutput=False)
    b_d = nc.declare_dram_parameter("bands", [128, 2, 4, 125], bf16, isOutput=False)
    w_d = nc.declare_dram_parameter("wfb", [128, 16], f32, isOutput=False)
    o_d = nc.declare_dram_parameter("out", [_C, _HO, _OPAD], bf16, isOutput=True)
    z_d = nc.declare_dram_parameter("zpad", [128, _CB, 4], bf16, isOutput=False)

    NBX = 4  # x-tile ring depth
    NBO = 4  # out-tile ring depth
    with tile.TileContext(nc) as tc:
        with (
            tc.tile_pool(name="sb", bufs=1) as pool,
            tc.tile_pool(name="ps", bufs=1, space="PSUM") as pp,
        ):
            band_sb = pool.tile([128, 2, 4, 125], bf16, tag="bands")
            nc.sync.dma_start(out=band_sb[:], in_=b_d[:])
            wf_sb = pool.tile([128, 16], f32, tag="wfb")
            nc.sync.dma_start(out=wf_sb[:], in_=w_d[:])

            # ---- gpsimd tail: out rows 250..256, channel-major ----
            xt2s, acc2s, ot2s = [], [], []
            for t in range(2):
                xt2 = pool.tile([128, 8, _NW], bf16, tag=f"xt2_{t}", name=f"xt2_{t}")
                nc.sync.dma_start(out=xt2[:, :, 0:2], in_=z_d[:, :, 0:2])
                nc.sync.dma_start(out=xt2[:, :, 258:_NW], in_=z_d[:, :, 0:4])
                nc.sync.dma_start(
                    out=xt2[:, :, 2:258],
                    in_=x_d[t * 128 : (t + 1) * 128, _TAIL_HLO : _TAIL_HLO + 8, :],
                )
                xt2s.append(xt2)
                acc2s.append(
                    pool.tile([128, 7, 2, _NMM], f32, tag=f"acc2_{t}", name=f"acc2_{t}")
                )
                ot2s.append(
                    pool.tile([128, 7, _OPAD], bf16, tag=f"ot2_{t}", name=f"ot2_{t}")
                )
            zeros = pool.tile([128, _NMM], f32, tag="zeros")
            nc.gpsimd.memset(zeros[:], 0.0)

            # Tail MACs run on DVE (the only engine with the fused
            # scalar_tensor_tensor op); they are emitted interleaved into the
            # main loop below so they don't delay the psum->sbuf copies that
            # gate PSUM reuse. Build the list of MAC thunks here.
            terms = _tail_terms()
            tail_macs = []
            tail_nmac = [[0] * 7 for _ in range(2)]
            for t in range(2):
                xt2, acc = xt2s[t], acc2s[t]
                for hp, hrow, i in terms:
                    for j in range(4):
                        k = tail_nmac[t][hp]

                        def mk(t=t, hp=hp, hrow=hrow, i=i, j=j, k=k):
                            src = (
                                zeros[:]
                                if k == 0
                                else acc2s[t][:, hp, (k + 1) % 2, :]
                            )
                            nc.vector.scalar_tensor_tensor(
                                out=acc2s[t][:, hp, k % 2, :],
                                in0=xt2s[t][:, hrow, j : j + _NMM],
                                scalar=wf_sb[:, i * 4 + j : i * 4 + j + 1],
                                in1=src,
                                op0=AO.mult,
                                op1=AO.add,
                            )

                        tail_macs.append(mk)
                        tail_nmac[t][hp] += 1

            def tail_finish():
                for t in range(2):
                    acc, ot2 = acc2s[t], ot2s[t]
                    for hp in range(7):
                        last = (tail_nmac[t][hp] + 1) % 2
                        nc.gpsimd.tensor_copy(
                            ot2[:, hp, 0:_WO], acc[:, hp, last, 0:_WO]
                        )
                    nc.sync.dma_start(
                        out=o_d[t * 128 : (t + 1) * 128, _TAIL_HP0:_HO, :],
                        in_=ot2s[t][:, :, :],
                    )

            # ---- PE main: out rows 0..249 ----
            xts = []
            for i in range(NBX):
                t = pool.tile([128, _CB, _NW], bf16, tag=f"xt{i}", name=f"xt{i}")
                nc.sync.dma_start(out=t[:, :, 0:2], in_=z_d[:, :, 0:2])
                nc.sync.dma_start(out=t[:, :, 258:_NW], in_=z_d[:, :, 0:4])
                xts.append(t)
            oss = [
                pool.tile([128, _CB, _OPAD], bf16, tag=f"os{i}", name=f"os{i}")
                for i in range(NBO)
            ]
            pss = [
                pp.tile([128, 4, 512], f32, tag=f"ps{i}", name=f"ps{i}")
                for i in range(2)
            ]

            it = 0
            for c0 in range(0, _C, _CB):
                for v, (hp0, Mv, hlo, Kv) in enumerate(_TILES):
                    xt = xts[it % NBX]
                    osb = oss[it % NBO]
                    nc.sync.dma_start(
                        out=xt[0:Kv, :, 2:258],
                        in_=x_d[c0 : c0 + _CB, hlo : hlo + Kv, :].rearrange(
                            "c h w -> h c w"
                        ),
                    )
                    for half in range(2):
                        ps = pss[half]
                        for j in range(4):
                            for ci in range(4):
                                cc = half * 4 + ci
                                nc.tensor.matmul(
                                    ps[0:Mv, ci, 0:_NMM],
                                    band_sb[0:Kv, v, j, 0:Mv],
                                    xt[0:Kv, cc, j : j + _NMM],
                                    start=(j == 0),
                                    stop=(j == 3),
                                )
                        if half == 0:
                            nc.vector.tensor_copy(
                                osb[0:Mv, 0:4, 0:_WO], ps[0:Mv, :, 0:_WO]
                            )
                        else:
                            nc.scalar.copy(
                                osb[0:Mv, 4:8, 0:_WO], ps[0:Mv, :, 0:_WO]
                            )
                    nc.scalar.dma_start(
                        out=o_d[c0 : c0 + _CB, hp0 : hp0 + Mv, :].rearrange(
                            "c h w -> h c w"
                        ),
                        in_=osb[0:Mv, :, 0:_OPAD],
                    )
                    for _ in range(4):
                        if tail_macs:
                            tail_macs.pop(0)()
                    it += 1
            while tail_macs:
                tail_macs.pop(0)()
            tail_finish()
    nc.finalize()
    _NC_CACHE["nc"] = nc
    return nc


def _run(x, kern, trace=False):
    import ml_dtypes
    from concourse.bass_utils import run_bass_kernel_spmd

    bf16 = ml_dtypes.bfloat16
    x = np.asarray(x, dtype=np.float32)
    wf = np.ascontiguousarray(np.asarray(kern, np.float32)[::-1, ::-1])
    bands = _build_bands(kern).astype(bf16)
    wfb = np.broadcast_to(wf.reshape(1, 16), (128, 16)).copy().astype(np.float32)
    nc = _build_nc()
    zpad = np.zeros((128, _CB, 4), bf16)
    in_maps = [
        {
            "x": np.ascontiguousarray(x[b]).astype(bf16),
            "bands": bands,
            "wfb": wfb,
            "zpad": zpad,
        }
        for b in range(_NCORES)
    ]
    res = run_bass_kernel_spmd(nc, in_maps, list(range(_NCORES)), trace=trace)
    out = np.stack(
        [
            np.asarray(res.results[i]["out"])[:, :, :_WO].astype(np.float32)
            for i in range(_NCORES)
        ],
        axis=0,
    )
    return out, res


def kernel(x, kernel):
    out, _ = _run(x, kernel, trace=False)
    return out


# revision 6
# speedup vs baseline: 1.4318x; 1.1385x over previous
"""Trainium2 Bass kernel for nn_Blur: depthwise 4x4 FIR conv, pad=2.

out[b,c,h',w'] = sum_{i,j} wf[i,j] * xpad[b,c,h'+i,w'+j],  wf = flip(kernel)
x: [8,256,256,256] f32, kernel: [4,4] f32 -> out: [8,256,257,257] f32

Strategy: pure data parallel over batch (8 cores, 1 batch elem each).
Per core, output rows 0..249 run on the TensorEngine as 4 banded-matrix
matmuls (one per kernel column j) accumulating in PSUM:
    psum[h', w'] += sum_h B_j[h,h'] * xpad_w[h, w'+j]
with B_j[h,h'] = wf[h-h'+2, j] built on the host from the runtime kernel.
The 7-row output tail (h' 250..256) runs channel-major on the gpsimd
engine as fused multiply-accumulate chains, so the PE only runs 2 full
125-row tiles per channel (the tail tile would cost a full matmul's
cycles for 7 rows of output).

Perf structure:
- bf16 end-to-end: x converted to bf16 on host, bands bf16, output
  stored bf16 and upcast on host. Halves HBM traffic (the memory-bound
  roofline); DMA lines stay >=512B so full DMA bus rate holds.
- PSUM as 2 tiles of [128, 4ch, 512] (bank-aligned channel stride):
  psum->sbuf copies batch 4 channels per instruction, and the two
  halves ping-pong so copies overlap the other half's matmuls.
- input DMA on SP ring, output DMA on ACT ring, copies split DVE/ACT.
- all weights (bands, tail scalars) are runtime DRAM inputs.
"""

import numpy as np

_C, _H, _W = 256, 256, 256
_HO, _WO = 257, 257
_NCORES = 8
# (hp0, Mv, hlo, Kv): output rows [hp0, hp0+Mv), contraction rows [hlo, hlo+Kv)
_TILES = [(0, 125, 0, 126), (125, 125, 123, 128)]
_TAIL_HP0 = 250  # output rows 250..256 on gpsimd
_TAIL_HLO = 248  # input rows 248..255
_NW = 262  # padded width in SBUF: 2 zero | 256 data | 4 zero
_NMM = 258  # matmul free dim (257 outputs + 1 garbage col)
_OPAD = 264  # padded output row pitch in DRAM (528B in bf16)
_CB = 8  # channels per DMA batch / psum rotation


def _build_bands(kern):
    wf = np.ascontiguousarray(np.asarray(kern, np.float32)[::-1, ::-1])
    bands = np.zeros((128, 2, 4, 128), np.float32)
    for v, (hp0, Mv, hlo, Kv) in enumerate(_TILES):
        for j in range(4):
            for hr in range(Kv):
                h = hlo + hr
                for mr in range(Mv):
                    i = h - (hp0 + mr) + 2
                    if 0 <= i < 4:
                        bands[hr, v, j, mr] = wf[i, j]
    return bands


def _tail_terms():
    """(hp, hrow, i) triples for the gpsimd tail: out row 250+hp uses
    input row 248+hrow with kernel row i."""
    terms = []
    for hp in range(7):
        for i in range(4):
            h = _TAIL_HP0 + hp + i - 2
            if _TAIL_HLO <= h < _H:
                terms.append((hp, h - _TAIL_HLO, i))
    return terms


_NC_CACHE = {}


def _build_nc():
    if "nc" in _NC_CACHE:
        return _NC_CACHE["nc"]
    import concourse.bacc as bacc
    import concourse.mybir as mybir
    import concourse.tile as tile

    bf16 = mybir.dt.bfloat16
    f32 = mybir.dt.float32
    AO = mybir.AluOpType
    nc = bacc.Bacc()
    x_d = nc.declare_dram_parameter("x", [_C, _H, _W], bf16, isOutput=False)
    b_d = nc.declare_dram_parameter("bands", [128, 2, 4, 128], bf16, isOutput=False)
    w_d = nc.declare_dram_parameter("wfb", [128, 16], f32, isOutput=False)
    o_d = nc.declare_dram_parameter("out", [_C, _HO, _OPAD], bf16, isOutput=True)
    z_d = nc.declare_dram_parameter("zpad", [128, _CB, 4], bf16, isOutput=False)

    NBX = 4  # x-tile ring depth
    NBO = 4  # out-tile ring depth
    with tile.TileContext(nc) as tc:
        with (
            tc.tile_pool(name="sb", bufs=1) as pool,
            tc.tile_pool(name="ps", bufs=1, space="PSUM") as pp,
        ):
            band_sb = pool.tile([128, 2, 4, 128], bf16, tag="bands")
            nc.sync.dma_start(out=band_sb[:], in_=b_d[:])
            wf_sb = pool.tile([128, 16], f32, tag="wfb")
            nc.sync.dma_start(out=wf_sb[:], in_=w_d[:])

            # ---- gpsimd tail: out rows 250..256, channel-major ----
            xt2s, acc2s, ot2s = [], [], []
            for t in range(2):
                xt2 = pool.tile([128, 8, _NW], bf16, tag=f"xt2_{t}", name=f"xt2_{t}")
                nc.sync.dma_start(out=xt2[:, :, 0:2], in_=z_d[:, :, 0:2])
                nc.sync.dma_start(out=xt2[:, :, 258:_NW], in_=z_d[:, :, 0:4])
                nc.sync.dma_start(
                    out=xt2[:, :, 2:258],
                    in_=x_d[t * 128 : (t + 1) * 128, _TAIL_HLO : _TAIL_HLO + 8, :],
                )
                xt2s.append(xt2)
                acc2s.append(
                    pool.tile([128, 7, 2, _NMM], f32, tag=f"acc2_{t}", name=f"acc2_{t}")
                )
                ot2s.append(
                    pool.tile([128, 7, _OPAD], bf16, tag=f"ot2_{t}", name=f"ot2_{t}")
                )
            zeros = pool.tile([128, _NMM], f32, tag="zeros")
            nc.gpsimd.memset(zeros[:], 0.0)

            # Tail MACs run on DVE (the only engine with the fused
            # scalar_tensor_tensor op); they are emitted interleaved into the
            # main loop below so they don't delay the psum->sbuf copies that
            # gate PSUM reuse. Build the list of MAC thunks here.
            terms = _tail_terms()
            tail_macs = []
            tail_nmac = [[0] * 7 for _ in range(2)]
            for t in range(2):
                xt2, acc = xt2s[t], acc2s[t]
                for hp, hrow, i in terms:
                    for j in range(4):
                        k = tail_nmac[t][hp]

                        def mk(t=t, hp=hp, hrow=hrow, i=i, j=j, k=k):
                            src = (
                                zeros[:]
                                if k == 0
                                else acc2s[t][:, hp, (k + 1) % 2, :]
                            )
                            nc.vector.scalar_tensor_tensor(
                                out=acc2s[t][:, hp, k % 2, :],
                                in0=xt2s[t][:, hrow, j : j + _NMM],
                                scalar=wf_sb[:, i * 4 + j : i * 4 + j + 1],
                                in1=src,
                                op0=AO.mult,
                                op1=AO.add,
                            )

                        tail_macs.append(mk)
                        tail_nmac[t][hp] += 1

            def tail_finish():
                for t in range(2):
                    acc, ot2 = acc2s[t], ot2s[t]
                    for hp in range(7):
                        last = (tail_nmac[t][hp] + 1) % 2
                        nc.gpsimd.tensor_copy(
                            ot2[:, hp, 0:_WO], acc[:, hp, last, 0:_WO]
                        )
                    nc.sync.dma_start(
                        out=o_d[t * 128 : (t + 1) * 128, _TAIL_HP0:_HO, :],
                        in_=ot2s[t][:, :, :],
                    )

            # ---- PE main: out rows 0..249 ----
            xts = []
            for i in range(NBX):
                t = pool.tile([128, _CB, _NW], bf16, tag=f"xt{i}", name=f"xt{i}")
                nc.sync.dma_start(out=t[:, :, 0:2], in_=z_d[:, :, 0:2])
                nc.sync.dma_start(out=t[:, :, 258:_NW], in_=z_d[:, :, 0:4])
                xts.append(t)
            oss = [
                pool.tile([128, _CB, _OPAD], bf16, tag=f"os{i}", name=f"os{i}")
                for i in range(NBO)
            ]
            pss = [
                pp.tile([128, 4, 512], f32, tag=f"ps{i}", name=f"ps{i}")
                for i in range(2)
            ]

            it = 0
            for c0 in range(0, _C, _CB):
                for v, (hp0, Mv, hlo, Kv) in enumerate(_TILES):
                    xt = xts[it % NBX]
                    osb = oss[it % NBO]
                    nc.sync.dma_start(
                        out=xt[0:Kv, :, 2:258],
                        in_=x_d[c0 : c0 + _CB, hlo : hlo + Kv, :].rearrange(
                            "c h w -> h c w"
                        ),
                    )
                    for half in range(2):
                        ps = pss[half]
                        for j in range(4):
                            for ci in range(4):
                                cc = half * 4 + ci
                                nc.tensor.matmul(
                                    ps[0:128, ci, 0:_NMM],
                                    band_sb[0:Kv, v, j, 0:128],
                                    xt[0:Kv, cc, j : j + _NMM],
                                    start=(j == 0),
                                    stop=(j == 3),
                                )
                        nc.scalar.copy(
                            osb[0:Mv, half * 4 : half * 4 + 4, 0:_WO],
                            ps[0:Mv, :, 0:_WO],
                        )
                    nc.gpsimd.dma_start(
                        out=o_d[c0 : c0 + _CB, hp0 : hp0 + Mv, :].rearrange(
                            "c h w -> h c w"
                        ),
                        in_=osb[0:Mv, :, 0:_OPAD],
                    )
                    for _ in range(4):
                        if tail_macs:
                            tail_macs.pop(0)()
                    it += 1
            while tail_macs:
                tail_macs.pop(0)()
            tail_finish()
    nc.finalize()
    _NC_CACHE["nc"] = nc
    return nc


def _run(x, kern, trace=False):
    import ml_dtypes
    from concourse.bass_utils import run_bass_kernel_spmd

    bf16 = ml_dtypes.bfloat16
    x = np.asarray(x, dtype=np.float32)
    wf = np.ascontiguousarray(np.asarray(kern, np.float32)[::-1, ::-1])
    bands = _build_bands(kern).astype(bf16)
    wfb = np.broadcast_to(wf.reshape(1, 16), (128, 16)).copy().astype(np.float32)
    nc = _build_nc()
    zpad = np.zeros((128, _CB, 4), bf16)
    in_maps = [
        {
            "x": np.ascontiguousarray(x[b]).astype(bf16),
            "bands": bands,
            "wfb": wfb,
            "zpad": zpad,
        }
        for b in range(_NCORES)
    ]
    res = run_bass_kernel_spmd(nc, in_maps, list(range(_NCORES)), trace=trace)
    out = np.stack(
        [
            np.asarray(res.results[i]["out"])[:, :, :_WO].astype(np.float32)
            for i in range(_NCORES)
        ],
        axis=0,
    )
    return out, res


def kernel(x, kernel):
    out, _ = _run(x, kernel, trace=False)
    return out


# revision 9
# speedup vs baseline: 2.1201x; 1.4807x over previous
"""Trainium2 Bass kernel for nn_Blur: depthwise 4x4 FIR conv, pad=2.

out[b,c,h',w'] = sum_{i,j} wf[i,j] * xpad[b,c,h'+i,w'+j],  wf = flip(kernel)
x: [8,256,256,256] f32, kernel: [4,4] f32 -> out: [8,256,257,257] f32

Strategy: pure data parallel over batch (8 cores, 1 batch elem each).
Per core, output rows 0..249 run on the TensorEngine as 4 banded-matrix
matmuls (one per kernel column j) accumulating in PSUM:
    psum[h', w'] += sum_h B_j[h,h'] * xpad_w[h, w'+j]
with B_j[h,h'] = wf[h-h'+2, j] built on the host from the runtime kernel.
The 7-row output tail (h' 250..256) runs channel-major on the DVE as
fused multiply-accumulate chains, so the PE only runs 2 full 125-row
tiles per channel.

Perf structure:
- bf16 end-to-end; x is host-transposed to [H, C, W] and the output is
  stored as [HO, C, OPAD]: every DMA line is then 8 channels x 512B
  contiguous (4KB), so each tile DMA is ~128 descriptors instead of
  ~1000 (descriptor generation on the rings was the v3 bottleneck).
- stationary band matrices padded to 128 columns (FWL eligibility).
- PSUM as 2 tiles of [128, 4ch, 512] (bank-aligned channel stride):
  psum->sbuf copies batch 4 channels per instruction and ping-pong so
  copies overlap the other half's matmuls. Copies split ACT/DVE.
- input DMA on SP ring, output DMA on Pool SWDGE ring.
- all weights (bands, tail scalars) are runtime DRAM inputs.
"""

import numpy as np

_C, _H, _W = 256, 256, 256
_HO, _WO = 257, 257
_NCORES = 8
# (hp0, Mv, hlo, Kv): output rows [hp0, hp0+Mv), contraction rows [hlo, hlo+Kv)
_TILES = [(0, 125, 0, 126), (125, 125, 123, 128)]
_TAIL_HP0 = 250  # output rows 250..256 on DVE
_TAIL_HLO = 248  # input rows 248..255
_NW = 262  # padded width in SBUF: 2 zero | 256 data | 4 zero
_NMM = 258  # matmul free dim (257 outputs + 1 garbage col)
_OPAD = 264  # padded output row pitch in DRAM (528B in bf16)
_CB = 8  # channels per DMA batch / psum rotation


def _build_bands(kern):
    wf = np.ascontiguousarray(np.asarray(kern, np.float32)[::-1, ::-1])
    bands = np.zeros((128, 2, 4, 128), np.float32)
    for v, (hp0, Mv, hlo, Kv) in enumerate(_TILES):
        for j in range(4):
            for hr in range(Kv):
                h = hlo + hr
                for mr in range(Mv):
                    i = h - (hp0 + mr) + 2
                    if 0 <= i < 4:
                        bands[hr, v, j, mr] = wf[i, j]
    return bands


def _tail_terms():
    """(hp, hrow, i) triples for the tail: out row 250+hp uses input row
    248+hrow with kernel row i."""
    terms = []
    for hp in range(7):
        for i in range(4):
            h = _TAIL_HP0 + hp + i - 2
            if _TAIL_HLO <= h < _H:
                terms.append((hp, h - _TAIL_HLO, i))
    return terms


_NC_CACHE = {}


def _build_nc():
    if "nc" in _NC_CACHE:
        return _NC_CACHE["nc"]
    import concourse.bacc as bacc
    import concourse.mybir as mybir
    import concourse.tile as tile

    bf16 = mybir.dt.bfloat16
    f32 = mybir.dt.float32
    AO = mybir.AluOpType
    nc = bacc.Bacc()
    # x, out live in [h][c][w] layout (host transposes)
    x_d = nc.declare_dram_parameter("x", [_H, _C, _W], bf16, isOutput=False)
    b_d = nc.declare_dram_parameter("bands", [128, 2, 4, 128], bf16, isOutput=False)
    w_d = nc.declare_dram_parameter("wfb", [128, 16], f32, isOutput=False)
    o_d = nc.declare_dram_parameter("out", [_HO, _C, _OPAD], bf16, isOutput=True)
    z_d = nc.declare_dram_parameter("zpad", [128, _CB, 4], bf16, isOutput=False)

    NBX = 4  # x-tile ring depth
    NBO = 4  # out-tile ring depth
    with tile.TileContext(nc) as tc:
        with (
            tc.tile_pool(name="sb", bufs=1) as pool,
            tc.tile_pool(name="ps", bufs=1, space="PSUM") as pp,
        ):
            band_sb = pool.tile([128, 2, 4, 128], bf16, tag="bands")
            nc.sync.dma_start(out=band_sb[:], in_=b_d[:])
            wf_sb = pool.tile([128, 16], f32, tag="wfb")
            nc.sync.dma_start(out=wf_sb[:], in_=w_d[:])

            # ---- tail tiles: out rows 250..256, channel-major ----
            xt2s, acc2s, ot2s = [], [], []
            for t in range(2):
                xt2 = pool.tile([128, 8, _NW], bf16, tag=f"xt2_{t}", name=f"xt2_{t}")
                nc.sync.dma_start(out=xt2[:, :, 0:2], in_=z_d[:, :, 0:2])
                nc.sync.dma_start(out=xt2[:, :, 258:_NW], in_=z_d[:, :, 0:4])
                nc.sync.dma_start(
                    out=xt2[:, :, 2:258],
                    in_=x_d[
                        _TAIL_HLO : _TAIL_HLO + 8, t * 128 : (t + 1) * 128, :
                    ].rearrange("h c w -> c h w"),
                )
                xt2s.append(xt2)
                acc2s.append(
                    pool.tile([128, 7, 2, _NMM], f32, tag=f"acc2_{t}", name=f"acc2_{t}")
                )
                ot2s.append(
                    pool.tile([128, 7, _OPAD], bf16, tag=f"ot2_{t}", name=f"ot2_{t}")
                )
            zeros = pool.tile([128, _NMM], f32, tag="zeros")
            nc.gpsimd.memset(zeros[:], 0.0)

            # Tail MACs on DVE (only engine with fused scalar_tensor_tensor);
            # emitted interleaved into the main loop so DVE stays out of the
            # psum-copy critical path. Each thunk is one MAC; after the last
            # MAC of a (t, hp) chain, the thunk also emits the bf16 copy, and
            # after the last chain of tile t, the tail output DMA.
            terms = _tail_terms()
            per_chain = {}
            for hp, hrow, i in terms:
                per_chain.setdefault(hp, []).append((hrow, i))
            tail_macs = []
            chains_done = {0: 0, 1: 0}
            for t in range(2):
                for hp in sorted(per_chain):
                    pairs = per_chain[hp]
                    nm = len(pairs) * 4

                    for k_idx in range(nm):
                        pi, j = divmod(k_idx, 4)
                        hrow, i = pairs[pi]

                        def mk(t=t, hp=hp, hrow=hrow, i=i, j=j, k=k_idx, nm=nm):
                            acc = acc2s[t]
                            src = (
                                zeros[:] if k == 0 else acc[:, hp, (k + 1) % 2, :]
                            )
                            nc.vector.scalar_tensor_tensor(
                                out=acc[:, hp, k % 2, :],
                                in0=xt2s[t][:, hrow, j : j + _NMM],
                                scalar=wf_sb[:, i * 4 + j : i * 4 + j + 1],
                                in1=src,
                                op0=AO.mult,
                                op1=AO.add,
                            )
                            if k == nm - 1:
                                nc.vector.tensor_copy(
                                    ot2s[t][:, hp, 0:_WO],
                                    acc[:, hp, (nm + 1) % 2, 0:_WO],
                                )
                                chains_done[t] += 1
                                if chains_done[t] == 7:
                                    nc.sync.dma_start(
                                        out=o_d[
                                            _TAIL_HP0:_HO, t * 128 : (t + 1) * 128, :
                                        ].rearrange("h c w -> c h w"),
                                        in_=ot2s[t][:, :, :],
                                    )

                        tail_macs.append(mk)

            # ---- PE main: out rows 0..249 ----
            xts = []
            for i in range(NBX):
                t = pool.tile([128, _CB, _NW], bf16, tag=f"xt{i}", name=f"xt{i}")
                nc.sync.dma_start(out=t[:, :, 0:2], in_=z_d[:, :, 0:2])
                nc.sync.dma_start(out=t[:, :, 258:_NW], in_=z_d[:, :, 0:4])
                xts.append(t)
            oss = [
                pool.tile([128, _CB, _OPAD], bf16, tag=f"os{i}", name=f"os{i}")
                for i in range(NBO)
            ]
            pss = [
                pp.tile([128, 4, 512], f32, tag=f"ps{i}", name=f"ps{i}")
                for i in range(2)
            ]

            it = 0
            for c0 in range(0, _C, _CB):
                for v, (hp0, Mv, hlo, Kv) in enumerate(_TILES):
                    xt = xts[it % NBX]
                    osb = oss[it % NBO]
                    nc.sync.dma_start(
                        out=xt[0:Kv, :, 2:258],
                        in_=x_d[hlo : hlo + Kv, c0 : c0 + _CB, :],
                    )
                    for half in range(2):
                        ps = pss[half]
                        for j in range(4):
                            for ci in range(4):
                                cc = half * 4 + ci
                                nc.tensor.matmul(
                                    ps[0:128, ci, 0:_NMM],
                                    band_sb[0:Kv, v, j, 0:128],
                                    xt[0:Kv, cc, j : j + _NMM],
                                    start=(j == 0),
                                    stop=(j == 3),
                                )
                        dst = osb[0:Mv, half * 4 : half * 4 + 4, 0:_WO]
                        src = ps[0:Mv, :, 0:_WO]
                        if half == 1 and it % 2 == 1:
                            nc.vector.tensor_copy(dst, src)
                        else:
                            nc.scalar.copy(dst, src)
                    nc.gpsimd.dma_start(
                        out=o_d[hp0 : hp0 + Mv, c0 : c0 + _CB, :],
                        in_=osb[0:Mv, :, 0:_OPAD],
                    )
                    for _ in range(4):
                        if tail_macs:
                            tail_macs.pop(0)()
                    it += 1
            while tail_macs:
                tail_macs.pop(0)()
    nc.finalize()
    _NC_CACHE["nc"] = nc
    return nc


def _run(x, kern, trace=False):
    import ml_dtypes
    from concourse.bass_utils import run_bass_kernel_spmd

    bf16 = ml_dtypes.bfloat16
    x = np.asarray(x, dtype=np.float32)
    wf = np.ascontiguousarray(np.asarray(kern, np.float32)[::-1, ::-1])
    bands = _build_bands(kern).astype(bf16)
    wfb = np.broadcast_to(wf.reshape(1, 16), (128, 16)).copy().astype(np.float32)
    nc = _build_nc()
    zpad = np.zeros((128, _CB, 4), bf16)
    in_maps = [
        {
            # [C,H,W] -> [H,C,W] so device DMA lines are 4KB contiguous
            "x": np.ascontiguousarray(x[b].transpose(1, 0, 2)).astype(bf16),
            "bands": bands,
            "wfb": wfb,
            "zpad": zpad,
        }
        for b in range(_NCORES)
    ]
    res = run_bass_kernel_spmd(nc, in_maps, list(range(_NCORES)), trace=trace)
    out = np.stack(
        [
            np.asarray(res.results[i]["out"])[:, :, :_WO]
            .transpose(1, 0, 2)
            .astype(np.float32)
            for i in range(_NCORES)
        ],
        axis=0,
    )
    return out, res


def kernel(x, kernel):
    out, _ = _run(x, kernel, trace=False)
    return out


# revision 11
# speedup vs baseline: 2.1806x; 1.0286x over previous
"""Trainium2 Bass kernel for nn_Blur: depthwise 4x4 FIR conv, pad=2.

out[b,c,h',w'] = sum_{i,j} wf[i,j] * xpad[b,c,h'+i,w'+j],  wf = flip(kernel)
x: [8,256,256,256] f32, kernel: [4,4] f32 -> out: [8,256,257,257] f32

Strategy: pure data parallel over batch (8 cores, 1 batch elem each).
Per core, output rows 0..249 run on the TensorEngine as 4 banded-matrix
matmuls (one per kernel column j) accumulating in PSUM:
    psum[h', w'] += sum_h B_j[h,h'] * xpad_w[h, w'+j]
with B_j[h,h'] = wf[h-h'+2, j] built on the host from the runtime kernel.
The 7-row output tail (h' 250..256) runs channel-major on the DVE as
fused multiply-accumulate chains, so the PE only runs 2 full 125-row
tiles per channel.

Perf structure:
- bf16 end-to-end; x is host-transposed to [H, C, W] and the output is
  stored as [HO, C, OPAD]: every DMA line is then 8 channels x 512B
  contiguous (4KB), so each tile DMA is ~128 descriptors instead of
  ~1000 (descriptor generation on the rings was the v3 bottleneck).
- stationary band matrices padded to 128 columns (FWL eligibility).
- PSUM as 2 tiles of [128, 4ch, 512] (bank-aligned channel stride):
  psum->sbuf copies batch 4 channels per instruction and ping-pong so
  copies overlap the other half's matmuls. Copies split ACT/DVE.
- input DMA on SP ring, output DMA on Pool SWDGE ring.
- all weights (bands, tail scalars) are runtime DRAM inputs.
"""

import numpy as np

_C, _H, _W = 256, 256, 256
_HO, _WO = 257, 257
_NCORES = 8
# (hp0, Mv, hlo, Kv): output rows [hp0, hp0+Mv), contraction rows [hlo, hlo+Kv)
_TILES = [(0, 125, 0, 126), (125, 125, 123, 128)]
_TAIL_HP0 = 250  # output rows 250..256 on DVE
_TAIL_HLO = 248  # input rows 248..255
_NW = 262  # padded width in SBUF: 2 zero | 256 data | 4 zero
_NMM = 258  # matmul free dim (257 outputs + 1 garbage col)
_OPAD = 264  # padded output row pitch in DRAM (528B in bf16)
_CB = 8  # channels per DMA batch / psum rotation


def _build_bands(kern):
    wf = np.ascontiguousarray(np.asarray(kern, np.float32)[::-1, ::-1])
    bands = np.zeros((128, 2, 4, 128), np.float32)
    for v, (hp0, Mv, hlo, Kv) in enumerate(_TILES):
        for j in range(4):
            for hr in range(Kv):
                h = hlo + hr
                for mr in range(Mv):
                    i = h - (hp0 + mr) + 2
                    if 0 <= i < 4:
                        bands[hr, v, j, mr] = wf[i, j]
    return bands


def _tail_terms():
    """(hp, hrow, i) triples for the tail: out row 250+hp uses input row
    248+hrow with kernel row i."""
    terms = []
    for hp in range(7):
        for i in range(4):
            h = _TAIL_HP0 + hp + i - 2
            if _TAIL_HLO <= h < _H:
                terms.append((hp, h - _TAIL_HLO, i))
    return terms


_NC_CACHE = {}


def _build_nc():
    if "nc" in _NC_CACHE:
        return _NC_CACHE["nc"]
    import concourse.bacc as bacc
    import concourse.mybir as mybir
    import concourse.tile as tile

    bf16 = mybir.dt.bfloat16
    f32 = mybir.dt.float32
    AO = mybir.AluOpType
    nc = bacc.Bacc()
    # x, out live in [h][c][w] layout (host transposes)
    x_d = nc.declare_dram_parameter("x", [_H, _C, _W], bf16, isOutput=False)
    b_d = nc.declare_dram_parameter("bands", [128, 2, 4, 128], bf16, isOutput=False)
    w_d = nc.declare_dram_parameter("wfb", [128, 16], f32, isOutput=False)
    o_d = nc.declare_dram_parameter("out", [_HO, _C, _OPAD], bf16, isOutput=True)
    z_d = nc.declare_dram_parameter("zpad", [128, _CB, 4], bf16, isOutput=False)

    NBX = 4  # x-tile ring depth
    NBO = 4  # out-tile ring depth
    with tile.TileContext(nc) as tc:
        with (
            tc.tile_pool(name="sb", bufs=1) as pool,
            tc.tile_pool(name="ps", bufs=1, space="PSUM") as pp,
        ):
            band_sb = pool.tile([128, 2, 4, 128], bf16, tag="bands")
            nc.sync.dma_start(out=band_sb[:], in_=b_d[:])
            wf_sb = pool.tile([128, 16], f32, tag="wfb")
            nc.sync.dma_start(out=wf_sb[:], in_=w_d[:])

            # ---- tail tiles: out rows 250..256, channel-major ----
            # Tail DMAs ride the Pool SWDGE ring so they don't delay the
            # first main input DMAs (SP/ACT rings) at startup.
            xt2s, acc2s, ot2s = [], [], []
            for t in range(2):
                xt2 = pool.tile([128, 8, _NW], bf16, tag=f"xt2_{t}", name=f"xt2_{t}")
                nc.gpsimd.dma_start(out=xt2[:, :, 0:2], in_=z_d[:, :, 0:2])
                nc.gpsimd.dma_start(out=xt2[:, :, 258:_NW], in_=z_d[:, :, 0:4])
                nc.gpsimd.dma_start(
                    out=xt2[:, :, 2:258],
                    in_=x_d[
                        _TAIL_HLO : _TAIL_HLO + 8, t * 128 : (t + 1) * 128, :
                    ].rearrange("h c w -> c h w"),
                )
                xt2s.append(xt2)
                acc2s.append(
                    pool.tile([128, 7, 2, _NMM], f32, tag=f"acc2_{t}", name=f"acc2_{t}")
                )
                ot2s.append(
                    pool.tile([128, 7, _OPAD], bf16, tag=f"ot2_{t}", name=f"ot2_{t}")
                )
            zeros = pool.tile([128, _NMM], f32, tag="zeros")
            nc.gpsimd.memset(zeros[:], 0.0)

            # Tail MACs on DVE (only engine with fused scalar_tensor_tensor);
            # emitted interleaved into the main loop so DVE stays out of the
            # psum-copy critical path. Each thunk is one MAC; after the last
            # MAC of a (t, hp) chain, the thunk also emits the bf16 copy, and
            # after the last chain of tile t, the tail output DMA.
            terms = _tail_terms()
            per_chain = {}
            for hp, hrow, i in terms:
                per_chain.setdefault(hp, []).append((hrow, i))
            tail_macs = []
            chains_done = {0: 0, 1: 0}
            for t in range(2):
                for hp in sorted(per_chain):
                    pairs = per_chain[hp]
                    nm = len(pairs) * 4

                    for k_idx in range(nm):
                        pi, j = divmod(k_idx, 4)
                        hrow, i = pairs[pi]

                        def mk(t=t, hp=hp, hrow=hrow, i=i, j=j, k=k_idx, nm=nm):
                            acc = acc2s[t]
                            src = (
                                zeros[:] if k == 0 else acc[:, hp, (k + 1) % 2, :]
                            )
                            nc.vector.scalar_tensor_tensor(
                                out=acc[:, hp, k % 2, :],
                                in0=xt2s[t][:, hrow, j : j + _NMM],
                                scalar=wf_sb[:, i * 4 + j : i * 4 + j + 1],
                                in1=src,
                                op0=AO.mult,
                                op1=AO.add,
                            )
                            if k == nm - 1:
                                nc.vector.tensor_copy(
                                    ot2s[t][:, hp, 0:_WO],
                                    acc[:, hp, (nm + 1) % 2, 0:_WO],
                                )
                                chains_done[t] += 1
                                if chains_done[t] == 7:
                                    nc.gpsimd.dma_start(
                                        out=o_d[
                                            _TAIL_HP0:_HO, t * 128 : (t + 1) * 128, :
                                        ].rearrange("h c w -> c h w"),
                                        in_=ot2s[t][:, :, :],
                                    )

                        tail_macs.append(mk)

            # ---- PE main: out rows 0..249 ----
            xts = [
                pool.tile([128, _CB, _NW], bf16, tag=f"xt{i}", name=f"xt{i}")
                for i in range(NBX)
            ]
            oss = [
                pool.tile([128, _CB, _OPAD], bf16, tag=f"os{i}", name=f"os{i}")
                for i in range(NBO)
            ]
            pss = [
                pp.tile([128, 4, 512], f32, tag=f"ps{i}", name=f"ps{i}")
                for i in range(2)
            ]

            it = 0
            for c0 in range(0, _C, _CB):
                for v, (hp0, Mv, hlo, Kv) in enumerate(_TILES):
                    xt = xts[it % NBX]
                    osb = oss[it % NBO]
                    ring = nc.sync if it % 2 == 0 else nc.scalar
                    if it < NBX:
                        ring.dma_start(out=xt[:, :, 0:2], in_=z_d[:, :, 0:2])
                        ring.dma_start(out=xt[:, :, 258:_NW], in_=z_d[:, :, 0:4])
                    ring.dma_start(
                        out=xt[0:Kv, :, 2:258],
                        in_=x_d[hlo : hlo + Kv, c0 : c0 + _CB, :],
                    )
                    for half in range(2):
                        ps = pss[half]
                        for j in range(4):
                            for ci in range(4):
                                cc = half * 4 + ci
                                nc.tensor.matmul(
                                    ps[0:128, ci, 0:_NMM],
                                    band_sb[0:Kv, v, j, 0:128],
                                    xt[0:Kv, cc, j : j + _NMM],
                                    start=(j == 0),
                                    stop=(j == 3),
                                )
                        dst = osb[0:Mv, half * 4 : half * 4 + 4, 0:_WO]
                        src = ps[0:Mv, :, 0:_WO]
                        if half == 1 and it % 2 == 1:
                            nc.vector.tensor_copy(dst, src)
                        else:
                            nc.scalar.copy(dst, src)
                    nc.gpsimd.dma_start(
                        out=o_d[hp0 : hp0 + Mv, c0 : c0 + _CB, :],
                        in_=osb[0:Mv, :, 0:_OPAD],
                    )
                    for _ in range(4):
                        if tail_macs:
                            tail_macs.pop(0)()
                    it += 1
            while tail_macs:
                tail_macs.pop(0)()
    nc.finalize()
    _NC_CACHE["nc"] = nc
    return nc


def _run(x, kern, trace=False):
    import ml_dtypes
    from concourse.bass_utils import run_bass_kernel_spmd

    bf16 = ml_dtypes.bfloat16
    x = np.asarray(x, dtype=np.float32)
    wf = np.ascontiguousarray(np.asarray(kern, np.float32)[::-1, ::-1])
    bands = _build_bands(kern).astype(bf16)
    wfb = np.broadcast_to(wf.reshape(1, 16), (128, 16)).copy().astype(np.float32)
    nc = _build_nc()
    zpad = np.zeros((128, _CB, 4), bf16)
    in_maps = [
        {
            # [C,H,W] -> [H,C,W] so device DMA lines are 4KB contiguous
            "x": np.ascontiguousarray(x[b].transpose(1, 0, 2)).astype(bf16),
            "bands": bands,
            "wfb": wfb,
            "zpad": zpad,
        }
        for b in range(_NCORES)
    ]
    res = run_bass_kernel_spmd(nc, in_maps, list(range(_NCORES)), trace=trace)
    out = np.stack(
        [
            np.asarray(res.results[i]["out"])[:, :, :_WO]
            .transpose(1, 0, 2)
            .astype(np.float32)
            for i in range(_NCORES)
        ],
        axis=0,
    )
    return out, res


def kernel(x, kernel):
    out, _ = _run(x, kernel, trace=False)
    return out


# revision 12
# speedup vs baseline: 2.2509x; 1.0322x over previous
"""Trainium2 Bass kernel for nn_Blur: depthwise 4x4 FIR conv, pad=2.

out[b,c,h',w'] = sum_{i,j} wf[i,j] * xpad[b,c,h'+i,w'+j],  wf = flip(kernel)
x: [8,256,256,256] f32, kernel: [4,4] f32 -> out: [8,256,257,257] f32

Strategy: pure data parallel over batch (8 cores, 1 batch elem each).
Per core, output rows 0..249 run on the TensorEngine as 4 banded-matrix
matmuls (one per kernel column j) accumulating in PSUM:
    psum[h', w'] += sum_h B_j[h,h'] * xpad_w[h, w'+j]
with B_j[h,h'] = wf[h-h'+2, j] built on the host from the runtime kernel.
The 7-row output tail (h' 250..256) runs channel-major on the DVE as
fused multiply-accumulate chains, so the PE only runs 2 full 125-row
tiles per channel.

Perf structure:
- bf16 end-to-end; x is host-transposed to [H, C, W] and the output is
  stored as [HO, C, OPAD]: every DMA line is then 8 channels x 512B
  contiguous (4KB), so each tile DMA is ~128 descriptors instead of
  ~1000 (descriptor generation on the rings was the v3 bottleneck).
- stationary band matrices padded to 128 columns (FWL eligibility).
- PSUM as 2 tiles of [128, 4ch, 512] (bank-aligned channel stride):
  psum->sbuf copies batch 4 channels per instruction and ping-pong so
  copies overlap the other half's matmuls. Copies split ACT/DVE.
- input DMA on SP ring, output DMA on Pool SWDGE ring.
- all weights (bands, tail scalars) are runtime DRAM inputs.
"""

import numpy as np

_C, _H, _W = 256, 256, 256
_HO, _WO = 257, 257
_NCORES = 8
# (hp0, Mv, hlo, Kv): output rows [hp0, hp0+Mv), contraction rows [hlo, hlo+Kv)
_TILES = [(0, 125, 0, 126), (125, 125, 123, 128)]
_TAIL_HP0 = 250  # output rows 250..256 on DVE
_TAIL_HLO = 248  # input rows 248..255
_NW = 262  # padded width in SBUF: 2 zero | 256 data | 4 zero
_NMM = 258  # matmul free dim (257 outputs + 1 garbage col)
_OPAD = 264  # padded output row pitch in DRAM (528B in bf16)
_CB = 8  # channels per DMA batch / psum rotation


def _build_bands(kern):
    wf = np.ascontiguousarray(np.asarray(kern, np.float32)[::-1, ::-1])
    bands = np.zeros((128, 2, 4, 128), np.float32)
    for v, (hp0, Mv, hlo, Kv) in enumerate(_TILES):
        for j in range(4):
            for hr in range(Kv):
                h = hlo + hr
                for mr in range(Mv):
                    i = h - (hp0 + mr) + 2
                    if 0 <= i < 4:
                        bands[hr, v, j, mr] = wf[i, j]
    return bands


def _tail_terms():
    """(hp, hrow, i) triples for the tail: out row 250+hp uses input row
    248+hrow with kernel row i."""
    terms = []
    for hp in range(7):
        for i in range(4):
            h = _TAIL_HP0 + hp + i - 2
            if _TAIL_HLO <= h < _H:
                terms.append((hp, h - _TAIL_HLO, i))
    return terms


_NC_CACHE = {}


def _build_nc():
    if "nc" in _NC_CACHE:
        return _NC_CACHE["nc"]
    import concourse.bacc as bacc
    import concourse.mybir as mybir
    import concourse.tile as tile

    bf16 = mybir.dt.bfloat16
    f32 = mybir.dt.float32
    AO = mybir.AluOpType
    nc = bacc.Bacc()
    # x, out live in [h][c][w] layout (host transposes)
    x_d = nc.declare_dram_parameter("x", [_H, _C, _NW], bf16, isOutput=False)
    b_d = nc.declare_dram_parameter("bands", [128, 2, 4, 128], bf16, isOutput=False)
    w_d = nc.declare_dram_parameter("wfb", [128, 16], f32, isOutput=False)
    o_d = nc.declare_dram_parameter("out", [_HO, _C, _OPAD], bf16, isOutput=True)

    NBX = 4  # x-tile ring depth
    NBO = 4  # out-tile ring depth
    with tile.TileContext(nc) as tc:
        with (
            tc.tile_pool(name="sb", bufs=1) as pool,
            tc.tile_pool(name="ps", bufs=1, space="PSUM") as pp,
        ):
            band_sb = pool.tile([128, 2, 4, 128], bf16, tag="bands")
            nc.scalar.dma_start(out=band_sb[:], in_=b_d[:])
            wf_sb = pool.tile([128, 16], f32, tag="wfb")
            nc.scalar.dma_start(out=wf_sb[:], in_=w_d[:])

            # ---- tail tiles: out rows 250..256, channel-major ----
            # Tail DMAs ride the Pool SWDGE ring so they don't delay the
            # first main input DMAs (SP/ACT rings) at startup.
            xt2s, acc2s, ot2s = [], [], []
            for t in range(2):
                xt2 = pool.tile([128, 8, _NW], bf16, tag=f"xt2_{t}", name=f"xt2_{t}")
                nc.gpsimd.dma_start(
                    out=xt2[:, :, :],
                    in_=x_d[
                        _TAIL_HLO : _TAIL_HLO + 8, t * 128 : (t + 1) * 128, :
                    ].rearrange("h c w -> c h w"),
                )
                xt2s.append(xt2)
                acc2s.append(
                    pool.tile([128, 7, 2, _NMM], f32, tag=f"acc2_{t}", name=f"acc2_{t}")
                )
                ot2s.append(
                    pool.tile([128, 7, _OPAD], bf16, tag=f"ot2_{t}", name=f"ot2_{t}")
                )
            zeros = pool.tile([128, _NMM], f32, tag="zeros")
            nc.gpsimd.memset(zeros[:], 0.0)

            # Tail MACs on DVE (only engine with fused scalar_tensor_tensor);
            # emitted interleaved into the main loop so DVE stays out of the
            # psum-copy critical path. Each thunk is one MAC; after the last
            # MAC of a (t, hp) chain, the thunk also emits the bf16 copy, and
            # after the last chain of tile t, the tail output DMA.
            terms = _tail_terms()
            per_chain = {}
            for hp, hrow, i in terms:
                per_chain.setdefault(hp, []).append((hrow, i))
            tail_macs = []
            chains_done = {0: 0, 1: 0}
            for t in range(2):
                for hp in sorted(per_chain):
                    pairs = per_chain[hp]
                    nm = len(pairs) * 4

                    for k_idx in range(nm):
                        pi, j = divmod(k_idx, 4)
                        hrow, i = pairs[pi]

                        def mk(t=t, hp=hp, hrow=hrow, i=i, j=j, k=k_idx, nm=nm):
                            acc = acc2s[t]
                            src = (
                                zeros[:] if k == 0 else acc[:, hp, (k + 1) % 2, :]
                            )
                            nc.vector.scalar_tensor_tensor(
                                out=acc[:, hp, k % 2, :],
                                in0=xt2s[t][:, hrow, j : j + _NMM],
                                scalar=wf_sb[:, i * 4 + j : i * 4 + j + 1],
                                in1=src,
                                op0=AO.mult,
                                op1=AO.add,
                            )
                            if k == nm - 1:
                                nc.vector.tensor_copy(
                                    ot2s[t][:, hp, 0:_WO],
                                    acc[:, hp, (nm + 1) % 2, 0:_WO],
                                )
                                chains_done[t] += 1
                                if chains_done[t] == 7:
                                    nc.gpsimd.dma_start(
                                        out=o_d[
                                            _TAIL_HP0:_HO, t * 128 : (t + 1) * 128, :
                                        ].rearrange("h c w -> c h w"),
                                        in_=ot2s[t][:, :, :],
                                    )

                        tail_macs.append(mk)

            # ---- PE main: out rows 0..249 ----
            xts = [
                pool.tile([128, _CB, _NW], bf16, tag=f"xt{i}", name=f"xt{i}")
                for i in range(NBX)
            ]
            oss = [
                pool.tile([128, _CB, _OPAD], bf16, tag=f"os{i}", name=f"os{i}")
                for i in range(NBO)
            ]
            pss = [
                pp.tile([128, 4, 512], f32, tag=f"ps{i}", name=f"ps{i}")
                for i in range(2)
            ]

            it = 0
            for c0 in range(0, _C, _CB):
                for v, (hp0, Mv, hlo, Kv) in enumerate(_TILES):
                    xt = xts[it % NBX]
                    osb = oss[it % NBO]
                    nc.sync.dma_start(
                        out=xt[0:Kv, :, :],
                        in_=x_d[hlo : hlo + Kv, c0 : c0 + _CB, :],
                    )
                    for half in range(2):
                        ps = pss[half]
                        for j in range(4):
                            for ci in range(4):
                                cc = half * 4 + ci
                                nc.tensor.matmul(
                                    ps[0:128, ci, 0:_NMM],
                                    band_sb[0:Kv, v, j, 0:128],
                                    xt[0:Kv, cc, j : j + _NMM],
                                    start=(j == 0),
                                    stop=(j == 3),
                                )
                        dst = osb[0:Mv, half * 4 : half * 4 + 4, 0:_WO]
                        src = ps[0:Mv, :, 0:_WO]
                        if half == 1 and it % 2 == 1:
                            nc.vector.tensor_copy(dst, src)
                        else:
                            nc.scalar.copy(dst, src)
                    nc.gpsimd.dma_start(
                        out=o_d[hp0 : hp0 + Mv, c0 : c0 + _CB, :],
                        in_=osb[0:Mv, :, 0:_OPAD],
                    )
                    for _ in range(4):
                        if tail_macs:
                            tail_macs.pop(0)()
                    it += 1
            while tail_macs:
                tail_macs.pop(0)()
    nc.finalize()
    _NC_CACHE["nc"] = nc
    return nc


def _run(x, kern, trace=False):
    import ml_dtypes
    from concourse.bass_utils import run_bass_kernel_spmd

    bf16 = ml_dtypes.bfloat16
    x = np.asarray(x, dtype=np.float32)
    wf = np.ascontiguousarray(np.asarray(kern, np.float32)[::-1, ::-1])
    bands = _build_bands(kern).astype(bf16)
    wfb = np.broadcast_to(wf.reshape(1, 16), (128, 16)).copy().astype(np.float32)
    nc = _build_nc()
    # [C,H,W] -> padded [H,C,262] bf16 (2 zero | 256 data | 4 zero) so every
    # device DMA line is one 4KB-contiguous run (128 descriptors per tile)
    in_maps = []
    for b in range(_NCORES):
        xp = np.zeros((_H, _C, _NW), bf16)
        xp[:, :, 2:258] = x[b].transpose(1, 0, 2).astype(bf16)
        in_maps.append({"x": xp, "bands": bands, "wfb": wfb})
    res = run_bass_kernel_spmd(nc, in_maps, list(range(_NCORES)), trace=trace)
    out = np.stack(
        [
            np.asarray(res.results[i]["out"])[:, :, :_WO]
            .transpose(1, 0, 2)
            .astype(np.float32)
            for i in range(_NCORES)
        ],
        axis=0,
    )
    return out, res


def kernel(x, kernel):
    out, _ = _run(x, kernel, trace=False)
    return out


# revision 13
# speedup vs baseline: 2.4957x; 1.1087x over previous
"""Trainium2 Bass kernel for nn_Blur: depthwise 4x4 FIR conv, pad=2.

out[b,c,h',w'] = sum_{i,j} wf[i,j] * xpad[b,c,h'+i,w'+j],  wf = flip(kernel)
x: [8,256,256,256] f32, kernel: [4,4] f32 -> out: [8,256,257,257] f32

Strategy: pure data parallel over batch (8 cores, 1 batch elem each).
Per core, output rows 0..249 run on the TensorEngine as 4 banded-matrix
matmuls (one per kernel column j) accumulating in PSUM:
    psum[h', w'] += sum_h B_j[h,h'] * xpad_w[h, w'+j]
with B_j[h,h'] = wf[h-h'+2, j] built on the host from the runtime kernel.
The 7-row output tail (h' 250..256) runs channel-major on the DVE as
fused multiply-accumulate chains, so the PE only runs 2 full 125-row
tiles per channel.

Perf structure:
- bf16 end-to-end; x is host-transposed to [H, C, W] and the output is
  stored as [HO, C, OPAD]: every DMA line is then 8 channels x 512B
  contiguous (4KB), so each tile DMA is ~128 descriptors instead of
  ~1000 (descriptor generation on the rings was the v3 bottleneck).
- stationary band matrices padded to 128 columns (FWL eligibility).
- PSUM as 2 tiles of [128, 4ch, 512] (bank-aligned channel stride):
  psum->sbuf copies batch 4 channels per instruction and ping-pong so
  copies overlap the other half's matmuls. Copies split ACT/DVE.
- input DMA on SP ring, output DMA on Pool SWDGE ring.
- all weights (bands, tail scalars) are runtime DRAM inputs.
"""

import numpy as np

_C, _H, _W = 256, 256, 256
_HO, _WO = 257, 257
_NCORES = 8
# (hp0, Mv, hlo, Kv): output rows [hp0, hp0+Mv), contraction rows [hlo, hlo+Kv)
_TILES = [(0, 125, 0, 126), (125, 125, 123, 128)]
_TAIL_HP0 = 250  # output rows 250..256 on DVE
_TAIL_HLO = 248  # input rows 248..255
_NW = 262  # padded width in SBUF: 2 zero | 256 data | 4 zero
_NMM = 258  # matmul free dim (257 outputs + 1 garbage col)
_OPAD = 264  # padded output row pitch in DRAM (528B in bf16)
_CB = 8  # channels per DMA batch / psum rotation


def _build_bands(kern):
    wf = np.ascontiguousarray(np.asarray(kern, np.float32)[::-1, ::-1])
    bands = np.zeros((128, 2, 4, 128), np.float32)
    for v, (hp0, Mv, hlo, Kv) in enumerate(_TILES):
        for j in range(4):
            for hr in range(Kv):
                h = hlo + hr
                for mr in range(Mv):
                    i = h - (hp0 + mr) + 2
                    if 0 <= i < 4:
                        bands[hr, v, j, mr] = wf[i, j]
    return bands


def _tail_terms():
    """(hp, hrow, i) triples for the tail: out row 250+hp uses input row
    248+hrow with kernel row i."""
    terms = []
    for hp in range(7):
        for i in range(4):
            h = _TAIL_HP0 + hp + i - 2
            if _TAIL_HLO <= h < _H:
                terms.append((hp, h - _TAIL_HLO, i))
    return terms


_NC_CACHE = {}


def _build_nc():
    if "nc" in _NC_CACHE:
        return _NC_CACHE["nc"]
    import concourse.bacc as bacc
    import concourse.mybir as mybir
    import concourse.tile as tile

    bf16 = mybir.dt.bfloat16
    f32 = mybir.dt.float32
    AO = mybir.AluOpType
    nc = bacc.Bacc()
    # x, out live in [h][c][w] layout (host transposes)
    x_d = nc.declare_dram_parameter("x", [_H, _C, _NW], bf16, isOutput=False)
    b_d = nc.declare_dram_parameter("bands", [128, 2, 4, 128], bf16, isOutput=False)
    w_d = nc.declare_dram_parameter("wfb", [128, 16], f32, isOutput=False)
    o_d = nc.declare_dram_parameter("out", [_HO, _C, _OPAD], bf16, isOutput=True)

    NBX = 6  # x-tile ring depth
    NBO = 6  # out-tile ring depth
    with tile.TileContext(nc) as tc:
        with (
            tc.tile_pool(name="sb", bufs=1) as pool,
            tc.tile_pool(name="ps", bufs=1, space="PSUM") as pp,
        ):
            band_sb = pool.tile([128, 2, 4, 128], bf16, tag="bands")
            nc.scalar.dma_start(out=band_sb[:], in_=b_d[:])
            wf_sb = pool.tile([128, 16], f32, tag="wfb")
            nc.scalar.dma_start(out=wf_sb[:], in_=w_d[:])

            # ---- tail tiles: out rows 250..256, channel-major ----
            # Tail DMAs ride the Pool SWDGE ring so they don't delay the
            # first main input DMAs (SP/ACT rings) at startup.
            xt2s, acc2s, ot2s = [], [], []
            for t in range(2):
                xt2 = pool.tile([128, 8, _NW], bf16, tag=f"xt2_{t}", name=f"xt2_{t}")
                nc.gpsimd.dma_start(
                    out=xt2[:, :, :],
                    in_=x_d[
                        _TAIL_HLO : _TAIL_HLO + 8, t * 128 : (t + 1) * 128, :
                    ].rearrange("h c w -> c h w"),
                )
                xt2s.append(xt2)
                acc2s.append(
                    pool.tile([128, 7, 2, _NMM], f32, tag=f"acc2_{t}", name=f"acc2_{t}")
                )
                ot2s.append(
                    pool.tile([128, 7, _OPAD], bf16, tag=f"ot2_{t}", name=f"ot2_{t}")
                )
            zeros = pool.tile([128, _NMM], f32, tag="zeros")
            nc.gpsimd.memset(zeros[:], 0.0)

            # Tail MACs on DVE (only engine with fused scalar_tensor_tensor);
            # emitted interleaved into the main loop so DVE stays out of the
            # psum-copy critical path. Each thunk is one MAC; after the last
            # MAC of a (t, hp) chain, the thunk also emits the bf16 copy, and
            # after the last chain of tile t, the tail output DMA.
            terms = _tail_terms()
            per_chain = {}
            for hp, hrow, i in terms:
                per_chain.setdefault(hp, []).append((hrow, i))
            tail_macs = []
            chains_done = {0: 0, 1: 0}
            for t in range(2):
                for hp in sorted(per_chain):
                    pairs = per_chain[hp]
                    nm = len(pairs) * 4

                    for k_idx in range(nm):
                        pi, j = divmod(k_idx, 4)
                        hrow, i = pairs[pi]

                        def mk(t=t, hp=hp, hrow=hrow, i=i, j=j, k=k_idx, nm=nm):
                            acc = acc2s[t]
                            src = (
                                zeros[:] if k == 0 else acc[:, hp, (k + 1) % 2, :]
                            )
                            nc.vector.scalar_tensor_tensor(
                                out=acc[:, hp, k % 2, :],
                                in0=xt2s[t][:, hrow, j : j + _NMM],
                                scalar=wf_sb[:, i * 4 + j : i * 4 + j + 1],
                                in1=src,
                                op0=AO.mult,
                                op1=AO.add,
                            )
                            if k == nm - 1:
                                nc.vector.tensor_copy(
                                    ot2s[t][:, hp, 0:_WO],
                                    acc[:, hp, (nm + 1) % 2, 0:_WO],
                                )
                                chains_done[t] += 1
                                if chains_done[t] == 7:
                                    nc.gpsimd.dma_start(
                                        out=o_d[
                                            _TAIL_HP0:_HO, t * 128 : (t + 1) * 128, :
                                        ].rearrange("h c w -> c h w"),
                                        in_=ot2s[t][:, :, :],
                                    )

                        tail_macs.append(mk)

            # ---- PE main: out rows 0..249 ----
            xts = [
                pool.tile([128, _CB, _NW], bf16, tag=f"xt{i}", name=f"xt{i}")
                for i in range(NBX)
            ]
            oss = [
                pool.tile([128, _CB, _OPAD], bf16, tag=f"os{i}", name=f"os{i}")
                for i in range(NBO)
            ]
            pss = [
                pp.tile([128, 4, 512], f32, tag=f"ps{i}", name=f"ps{i}")
                for i in range(2)
            ]

            it = 0
            for c0 in range(0, _C, _CB):
                for v, (hp0, Mv, hlo, Kv) in enumerate(_TILES):
                    xt = xts[it % NBX]
                    osb = oss[it % NBO]
                    nc.sync.dma_start(
                        out=xt[0:Kv, :, :],
                        in_=x_d[hlo : hlo + Kv, c0 : c0 + _CB, :],
                    )
                    for half in range(2):
                        ps = pss[half]
                        for j in range(4):
                            for ci in range(4):
                                cc = half * 4 + ci
                                nc.tensor.matmul(
                                    ps[0:128, ci, 0:_NMM],
                                    band_sb[0:Kv, v, j, 0:128],
                                    xt[0:Kv, cc, j : j + _NMM],
                                    start=(j == 0),
                                    stop=(j == 3),
                                )
                        nc.scalar.copy(
                            osb[0:Mv, half * 4 : half * 4 + 4, 0:_WO],
                            ps[0:Mv, :, 0:_WO],
                        )
                    nc.gpsimd.dma_start(
                        out=o_d[hp0 : hp0 + Mv, c0 : c0 + _CB, :],
                        in_=osb[0:Mv, :, 0:_OPAD],
                    )
                    for _ in range(4):
                        if tail_macs:
                            tail_macs.pop(0)()
                    it += 1
            while tail_macs:
                tail_macs.pop(0)()
    nc.finalize()
    _NC_CACHE["nc"] = nc
    return nc


def _run(x, kern, trace=False):
    import ml_dtypes
    from concourse.bass_utils import run_bass_kernel_spmd

    bf16 = ml_dtypes.bfloat16
    x = np.asarray(x, dtype=np.float32)
    wf = np.ascontiguousarray(np.asarray(kern, np.float32)[::-1, ::-1])
    bands = _build_bands(kern).astype(bf16)
    wfb = np.broadcast_to(wf.reshape(1, 16), (128, 16)).copy().astype(np.float32)
    nc = _build_nc()
    # [C,H,W] -> padded [H,C,262] bf16 (2 zero | 256 data | 4 zero) so every
    # device DMA line is one 4KB-contiguous run (128 descriptors per tile)
    in_maps = []
    for b in range(_NCORES):
        xp = np.zeros((_H, _C, _NW), bf16)
        xp[:, :, 2:258] = x[b].transpose(1, 0, 2).astype(bf16)
        in_maps.append({"x": xp, "bands": bands, "wfb": wfb})
    res = run_bass_kernel_spmd(nc, in_maps, list(range(_NCORES)), trace=trace)
    out = np.stack(
        [
            np.asarray(res.results[i]["out"])[:, :, :_WO]
            .transpose(1, 0, 2)
            .astype(np.float32)
            for i in range(_NCORES)
        ],
        axis=0,
    )
    return out, res


def kernel(x, kernel):
    out, _ = _run(x, kernel, trace=False)
    return out


# revision 15
# speedup vs baseline: 2.5339x; 1.0153x over previous
"""Trainium2 Bass kernel for nn_Blur: depthwise 4x4 FIR conv, pad=2.

out[b,c,h',w'] = sum_{i,j} wf[i,j] * xpad[b,c,h'+i,w'+j],  wf = flip(kernel)
x: [8,256,256,256] f32, kernel: [4,4] f32 -> out: [8,256,257,257] f32

Strategy: pure data parallel over batch (8 cores, 1 batch elem each).
Per core, output rows 0..249 run on the TensorEngine as 4 banded-matrix
matmuls (one per kernel column j) accumulating in PSUM:
    psum[h', w'] += sum_h B_j[h,h'] * xpad_w[h, w'+j]
with B_j[h,h'] = wf[h-h'+2, j] built on the host from the runtime kernel.
The 7-row output tail (h' 250..256) runs channel-major on the DVE as
fused multiply-accumulate chains, so the PE only runs 2 full 125-row
tiles per channel.

Perf structure:
- bf16 end-to-end; x is host-transposed to [H, C, W] and the output is
  stored as [HO, C, OPAD]: every DMA line is then 8 channels x 512B
  contiguous (4KB), so each tile DMA is ~128 descriptors instead of
  ~1000 (descriptor generation on the rings was the v3 bottleneck).
- stationary band matrices padded to 128 columns (FWL eligibility).
- PSUM as 2 tiles of [128, 4ch, 512] (bank-aligned channel stride):
  psum->sbuf copies batch 4 channels per instruction and ping-pong so
  copies overlap the other half's matmuls. Copies split ACT/DVE.
- input DMA on SP ring, output DMA on Pool SWDGE ring.
- all weights (bands, tail scalars) are runtime DRAM inputs.
"""

import numpy as np

_C, _H, _W = 256, 256, 256
_HO, _WO = 257, 257
_NCORES = 8
# (hp0, Mv, hlo, Kv): output rows [hp0, hp0+Mv), contraction rows [hlo, hlo+Kv)
_TILES = [(0, 125, 0, 126), (125, 125, 123, 128)]
_TAIL_HP0 = 250  # output rows 250..256 on DVE
_TAIL_HLO = 248  # input rows 248..255
_NW = 262  # padded width in SBUF: 2 zero | 256 data | 4 zero
_NMM = 258  # matmul free dim (257 outputs + 1 garbage col)
_OPAD = 264  # padded output row pitch in DRAM (528B in bf16)
_CB = 8  # channels per DMA batch / psum rotation


def _build_bands(kern):
    wf = np.ascontiguousarray(np.asarray(kern, np.float32)[::-1, ::-1])
    bands = np.zeros((128, 2, 4, 128), np.float32)
    for v, (hp0, Mv, hlo, Kv) in enumerate(_TILES):
        for j in range(4):
            for hr in range(Kv):
                h = hlo + hr
                for mr in range(Mv):
                    i = h - (hp0 + mr) + 2
                    if 0 <= i < 4:
                        bands[hr, v, j, mr] = wf[i, j]
    return bands


def _tail_terms():
    """(hp, hrow, i) triples for the tail: out row 250+hp uses input row
    248+hrow with kernel row i."""
    terms = []
    for hp in range(7):
        for i in range(4):
            h = _TAIL_HP0 + hp + i - 2
            if _TAIL_HLO <= h < _H:
                terms.append((hp, h - _TAIL_HLO, i))
    return terms


_NC_CACHE = {}


def _build_nc():
    if "nc" in _NC_CACHE:
        return _NC_CACHE["nc"]
    import concourse.bacc as bacc
    import concourse.mybir as mybir
    import concourse.tile as tile

    bf16 = mybir.dt.bfloat16
    f32 = mybir.dt.float32
    AO = mybir.AluOpType
    nc = bacc.Bacc()
    # x, out live in [h][c][w] layout (host transposes)
    x_d = nc.declare_dram_parameter("x", [_H, _C, _NW], bf16, isOutput=False)
    b_d = nc.declare_dram_parameter("bands", [128, 2, 4, 128], bf16, isOutput=False)
    w_d = nc.declare_dram_parameter("wfb", [128, 16], f32, isOutput=False)
    o_d = nc.declare_dram_parameter("out", [_HO, _C, _OPAD], bf16, isOutput=True)

    NBX = 6  # x-tile ring depth
    NBO = 6  # out-tile ring depth
    with tile.TileContext(nc) as tc:
        with (
            tc.tile_pool(name="sb", bufs=1) as pool,
            tc.tile_pool(name="ps", bufs=1, space="PSUM") as pp,
        ):
            band_sb = pool.tile([128, 2, 4, 128], bf16, tag="bands")
            nc.scalar.dma_start(out=band_sb[:], in_=b_d[:])
            wf_sb = pool.tile([128, 16], f32, tag="wfb")
            nc.scalar.dma_start(out=wf_sb[:], in_=w_d[:])

            # ---- tail tiles: out rows 250..256, channel-major ----
            # Tail DMAs ride the Pool SWDGE ring so they don't delay the
            # first main input DMAs (SP/ACT rings) at startup.
            xt2s, acc2s, ot2s = [], [], []
            for t in range(2):
                xt2 = pool.tile([128, 8, _NW], bf16, tag=f"xt2_{t}", name=f"xt2_{t}")
                nc.gpsimd.dma_start(
                    out=xt2[:, :, :],
                    in_=x_d[
                        _TAIL_HLO : _TAIL_HLO + 8, t * 128 : (t + 1) * 128, :
                    ].rearrange("h c w -> c h w"),
                )
                xt2s.append(xt2)
                acc2s.append(
                    pool.tile([128, 7, 2, _NMM], f32, tag=f"acc2_{t}", name=f"acc2_{t}")
                )
                ot2s.append(
                    pool.tile([128, 7, _OPAD], bf16, tag=f"ot2_{t}", name=f"ot2_{t}")
                )
            zeros = pool.tile([128, _NMM], f32, tag="zeros")
            nc.gpsimd.memset(zeros[:], 0.0)

            # Tail MACs on DVE (only engine with fused scalar_tensor_tensor);
            # emitted interleaved into the main loop so DVE stays out of the
            # psum-copy critical path. Each thunk is one MAC; after the last
            # MAC of a (t, hp) chain, the thunk also emits the bf16 copy, and
            # after the last chain of tile t, the tail output DMA.
            terms = _tail_terms()
            per_chain = {}
            for hp, hrow, i in terms:
                per_chain.setdefault(hp, []).append((hrow, i))
            tail_macs = []
            chains_done = {0: 0, 1: 0}
            for t in range(2):
                for hp in sorted(per_chain):
                    pairs = per_chain[hp]
                    nm = len(pairs) * 4

                    for k_idx in range(nm):
                        pi, j = divmod(k_idx, 4)
                        hrow, i = pairs[pi]

                        def mk(t=t, hp=hp, hrow=hrow, i=i, j=j, k=k_idx, nm=nm):
                            acc = acc2s[t]
                            src = (
                                zeros[:] if k == 0 else acc[:, hp, (k + 1) % 2, :]
                            )
                            nc.vector.scalar_tensor_tensor(
                                out=acc[:, hp, k % 2, :],
                                in0=xt2s[t][:, hrow, j : j + _NMM],
                                scalar=wf_sb[:, i * 4 + j : i * 4 + j + 1],
                                in1=src,
                                op0=AO.mult,
                                op1=AO.add,
                            )
                            if k == nm - 1:
                                nc.vector.tensor_copy(
                                    ot2s[t][:, hp, 0:_WO],
                                    acc[:, hp, (nm + 1) % 2, 0:_WO],
                                )
                                chains_done[t] += 1
                                if chains_done[t] == 7:
                                    nc.gpsimd.dma_start(
                                        out=o_d[
                                            _TAIL_HP0:_HO, t * 128 : (t + 1) * 128, :
                                        ].rearrange("h c w -> c h w"),
                                        in_=ot2s[t][:, :, :],
                                    )

                        tail_macs.append(mk)

            # ---- PE main: out rows 0..249 ----
            xts = [
                pool.tile([128, _CB, _NW], bf16, tag=f"xt{i}", name=f"xt{i}")
                for i in range(NBX)
            ]
            oss = [
                pool.tile([128, _CB, _OPAD], bf16, tag=f"os{i}", name=f"os{i}")
                for i in range(NBO)
            ]
            pss = [
                pp.tile([128, 4, 512], f32, tag=f"ps{i}", name=f"ps{i}")
                for i in range(2)
            ]

            it = 0
            for c0 in range(0, _C, _CB):
                for v, (hp0, Mv, hlo, Kv) in enumerate(_TILES):
                    xt = xts[it % NBX]
                    osb = oss[it % NBO]
                    nc.sync.dma_start(
                        out=xt[0:Kv, :, :],
                        in_=x_d[hlo : hlo + Kv, c0 : c0 + _CB, :],
                    )
                    for half in range(2):
                        ps = pss[half]
                        for j in range(4):
                            for ci in range(4):
                                cc = half * 4 + ci
                                nc.tensor.matmul(
                                    ps[0:128, ci, 0:_NMM],
                                    band_sb[0:Kv, v, j, 0:128],
                                    xt[0:Kv, cc, j : j + _NMM],
                                    start=(j == 0),
                                    stop=(j == 3),
                                )
                        nc.scalar.copy(
                            osb[0:Mv, half * 4 : half * 4 + 4, 0:_WO],
                            ps[0:Mv, :, 0:_WO],
                        )
                    nc.gpsimd.dma_start(
                        out=o_d[hp0 : hp0 + Mv, c0 : c0 + _CB, :],
                        in_=osb[0:Mv, :, 0:_OPAD],
                    )
                    for _ in range(4):
                        if tail_macs:
                            tail_macs.pop(0)()
                    it += 1
            while tail_macs:
                tail_macs.pop(0)()
    nc.finalize()
    _NC_CACHE["nc"] = nc
    return nc


def _run(x, kern, trace=False):
    import ml_dtypes
    from concourse.bass_utils import run_bass_kernel_spmd

    bf16 = ml_dtypes.bfloat16
    x = np.asarray(x, dtype=np.float32)
    wf = np.ascontiguousarray(np.asarray(kern, np.float32)[::-1, ::-1])
    bands = _build_bands(kern).astype(bf16)
    wfb = np.broadcast_to(wf.reshape(1, 16), (128, 16)).copy().astype(np.float32)
    nc = _build_nc()
    # [C,H,W] -> padded [H,C,262] bf16 (2 zero | 256 data | 4 zero) so every
    # device DMA line is one 4KB-contiguous run (128 descriptors per tile)
    in_maps = []
    for b in range(_NCORES):
        xp = np.zeros((_H, _C, _NW), bf16)
        xp[:, :, 2:258] = x[b].transpose(1, 0, 2).astype(bf16)
        in_maps.append({"x": xp, "bands": bands, "wfb": wfb})
    res = run_bass_kernel_spmd(nc, in_maps, list(range(_NCORES)), trace=trace)
    out = np.stack(
        [
            np.asarray(res.results[i]["out"])[:, :, :_WO]
            .transpose(1, 0, 2)
            .astype(np.float32)
            for i in range(_NCORES)
        ],
        axis=0,
    )
    return out, res


def kernel(x, kernel):
    out, _ = _run(x, kernel, trace=False)
    return out


# revision 16
# speedup vs baseline: 2.6215x; 1.0346x over previous
"""Trainium2 Bass kernel for nn_Blur: depthwise 4x4 FIR conv, pad=2.

out[b,c,h',w'] = sum_{i,j} wf[i,j] * xpad[b,c,h'+i,w'+j],  wf = flip(kernel)
x: [8,256,256,256] f32, kernel: [4,4] f32 -> out: [8,256,257,257] f32

Strategy: pure data parallel over batch (8 cores, 1 batch elem each).
Per core, output rows 0..249 run on the TensorEngine as 4 banded-matrix
matmuls (one per kernel column j) accumulating in PSUM:
    psum[h', w'] += sum_h B_j[h,h'] * xpad_w[h, w'+j]
with B_j[h,h'] = wf[h-h'+2, j] built on the host from the runtime kernel.
The 7-row output tail (h' 250..256) runs channel-major on the DVE as
fused multiply-accumulate chains, so the PE only runs 2 full 125-row
tiles per channel.

Perf structure:
- bf16 end-to-end; x is host-transposed to [H, C, W] and the output is
  stored as [HO, C, OPAD]: every DMA line is then 8 channels x 512B
  contiguous (4KB), so each tile DMA is ~128 descriptors instead of
  ~1000 (descriptor generation on the rings was the v3 bottleneck).
- stationary band matrices padded to 128 columns (FWL eligibility).
- PSUM as 2 tiles of [128, 4ch, 512] (bank-aligned channel stride):
  psum->sbuf copies batch 4 channels per instruction and ping-pong so
  copies overlap the other half's matmuls. Copies split ACT/DVE.
- input DMA on SP ring, output DMA on Pool SWDGE ring.
- all weights (bands, tail scalars) are runtime DRAM inputs.
"""

import numpy as np

_C, _H, _W = 256, 256, 256
_HO, _WO = 257, 257
_NCORES = 8
# (hp0, Mv, hlo, Kv): output rows [hp0, hp0+Mv), contraction rows [hlo, hlo+Kv)
_TILES = [(0, 125, 0, 126), (125, 125, 123, 128)]
_TAIL_HP0 = 250  # output rows 250..256 on DVE
_TAIL_HLO = 248  # input rows 248..255
_NW = 262  # padded width in SBUF: 2 zero | 256 data | 4 zero
_NMM = 258  # matmul free dim (257 outputs + 1 garbage col)
_OPAD = 264  # padded output row pitch in DRAM (528B in bf16)
_CB = 8  # channels per DMA batch / psum rotation


def _build_bands(kern):
    wf = np.ascontiguousarray(np.asarray(kern, np.float32)[::-1, ::-1])
    bands = np.zeros((128, 2, 4, 128), np.float32)
    for v, (hp0, Mv, hlo, Kv) in enumerate(_TILES):
        for j in range(4):
            for hr in range(Kv):
                h = hlo + hr
                for mr in range(Mv):
                    i = h - (hp0 + mr) + 2
                    if 0 <= i < 4:
                        bands[hr, v, j, mr] = wf[i, j]
    return bands


def _tail_terms():
    """(hp, hrow, i) triples for the tail: out row 250+hp uses input row
    248+hrow with kernel row i."""
    terms = []
    for hp in range(7):
        for i in range(4):
            h = _TAIL_HP0 + hp + i - 2
            if _TAIL_HLO <= h < _H:
                terms.append((hp, h - _TAIL_HLO, i))
    return terms


_NC_CACHE = {}


def _build_nc():
    if "nc" in _NC_CACHE:
        return _NC_CACHE["nc"]
    import concourse.bacc as bacc
    import concourse.mybir as mybir
    import concourse.tile as tile

    bf16 = mybir.dt.bfloat16
    f32 = mybir.dt.float32
    AO = mybir.AluOpType
    nc = bacc.Bacc()
    # x, out live in [h][c][w] layout (host transposes)
    x_d = nc.declare_dram_parameter("x", [_H, _C, _NW], bf16, isOutput=False)
    b_d = nc.declare_dram_parameter("bands", [128, 2, 4, 128], bf16, isOutput=False)
    w_d = nc.declare_dram_parameter("wfb", [128, 16], f32, isOutput=False)
    o_d = nc.declare_dram_parameter("out", [_HO, _C, _OPAD], bf16, isOutput=True)

    NBX = 8  # x-tile ring depth
    NBO = 12  # out-tile ring depth
    with tile.TileContext(nc) as tc:
        with (
            tc.tile_pool(name="sb", bufs=1) as pool,
            tc.tile_pool(name="ps", bufs=1, space="PSUM") as pp,
        ):
            band_sb = pool.tile([128, 2, 4, 128], bf16, tag="bands")
            nc.scalar.dma_start(out=band_sb[:], in_=b_d[:])
            wf_sb = pool.tile([128, 16], f32, tag="wfb")
            nc.scalar.dma_start(out=wf_sb[:], in_=w_d[:])

            # ---- tail tiles: out rows 250..256, channel-major ----
            # Tail DMAs ride the Pool SWDGE ring so they don't delay the
            # first main input DMAs (SP/ACT rings) at startup.
            xt2s, acc2s, ot2s = [], [], []
            for t in range(2):
                xt2 = pool.tile([128, 8, _NW], bf16, tag=f"xt2_{t}", name=f"xt2_{t}")
                nc.gpsimd.dma_start(
                    out=xt2[:, :, :],
                    in_=x_d[
                        _TAIL_HLO : _TAIL_HLO + 8, t * 128 : (t + 1) * 128, :
                    ].rearrange("h c w -> c h w"),
                )
                xt2s.append(xt2)
                acc2s.append(
                    pool.tile([128, 7, 2, _NMM], f32, tag=f"acc2_{t}", name=f"acc2_{t}")
                )
                ot2s.append(
                    pool.tile([128, 7, _OPAD], bf16, tag=f"ot2_{t}", name=f"ot2_{t}")
                )
            zeros = pool.tile([128, _NMM], f32, tag="zeros")
            nc.gpsimd.memset(zeros[:], 0.0)

            # Tail MACs on DVE (only engine with fused scalar_tensor_tensor);
            # emitted interleaved into the main loop so DVE stays out of the
            # psum-copy critical path. Each thunk is one MAC; after the last
            # MAC of a (t, hp) chain, the thunk also emits the bf16 copy, and
            # after the last chain of tile t, the tail output DMA.
            terms = _tail_terms()
            per_chain = {}
            for hp, hrow, i in terms:
                per_chain.setdefault(hp, []).append((hrow, i))
            tail_macs = []
            chains_done = {0: 0, 1: 0}
            for t in range(2):
                for hp in sorted(per_chain):
                    pairs = per_chain[hp]
                    nm = len(pairs) * 4

                    for k_idx in range(nm):
                        pi, j = divmod(k_idx, 4)
                        hrow, i = pairs[pi]

                        def mk(t=t, hp=hp, hrow=hrow, i=i, j=j, k=k_idx, nm=nm):
                            acc = acc2s[t]
                            src = (
                                zeros[:] if k == 0 else acc[:, hp, (k + 1) % 2, :]
                            )
                            nc.vector.scalar_tensor_tensor(
                                out=acc[:, hp, k % 2, :],
                                in0=xt2s[t][:, hrow, j : j + _NMM],
                                scalar=wf_sb[:, i * 4 + j : i * 4 + j + 1],
                                in1=src,
                                op0=AO.mult,
                                op1=AO.add,
                            )
                            if k == nm - 1:
                                nc.vector.tensor_copy(
                                    ot2s[t][:, hp, 0:_WO],
                                    acc[:, hp, (nm + 1) % 2, 0:_WO],
                                )
                                chains_done[t] += 1
                                if chains_done[t] == 7:
                                    nc.gpsimd.dma_start(
                                        out=o_d[
                                            _TAIL_HP0:_HO, t * 128 : (t + 1) * 128, :
                                        ].rearrange("h c w -> c h w"),
                                        in_=ot2s[t][:, :, :],
                                    )

                        tail_macs.append(mk)

            # ---- PE main: out rows 0..249 ----
            xts = [
                pool.tile([128, _CB, _NW], bf16, tag=f"xt{i}", name=f"xt{i}")
                for i in range(NBX)
            ]
            oss = [
                pool.tile([128, _CB, _OPAD], bf16, tag=f"os{i}", name=f"os{i}")
                for i in range(NBO)
            ]
            pss = [
                pp.tile([128, 4, 512], f32, tag=f"ps{i}", name=f"ps{i}")
                for i in range(2)
            ]

            it = 0
            for c0 in range(0, _C, _CB):
                for v, (hp0, Mv, hlo, Kv) in enumerate(_TILES):
                    xt = xts[it % NBX]
                    osb = oss[it % NBO]
                    nc.sync.dma_start(
                        out=xt[0:Kv, :, :],
                        in_=x_d[hlo : hlo + Kv, c0 : c0 + _CB, :],
                    )
                    for half in range(2):
                        ps = pss[half]
                        for j in range(4):
                            for ci in range(4):
                                cc = half * 4 + ci
                                nc.tensor.matmul(
                                    ps[0:128, ci, 0:_NMM],
                                    band_sb[0:Kv, v, j, 0:128],
                                    xt[0:Kv, cc, j : j + _NMM],
                                    start=(j == 0),
                                    stop=(j == 3),
                                )
                        dst = osb[0:Mv, half * 4 : half * 4 + 4, 0:_WO]
                        src_ = ps[0:Mv, :, 0:_WO]
                        # late tiles: DVE is past its tail MACs, split the
                        # copy pair across ACT+DVE to shorten the drain
                        if half == 1 and it >= 52:
                            nc.vector.tensor_copy(dst, src_)
                        else:
                            nc.scalar.copy(dst, src_)
                    nc.gpsimd.dma_start(
                        out=o_d[hp0 : hp0 + Mv, c0 : c0 + _CB, :],
                        in_=osb[0:Mv, :, 0:_OPAD],
                    )
                    for _ in range(4):
                        if tail_macs:
                            tail_macs.pop(0)()
                    it += 1
            while tail_macs:
                tail_macs.pop(0)()
    nc.finalize()
    _NC_CACHE["nc"] = nc
    return nc


def _run(x, kern, trace=False):
    import ml_dtypes
    from concourse.bass_utils import run_bass_kernel_spmd

    bf16 = ml_dtypes.bfloat16
    x = np.asarray(x, dtype=np.float32)
    wf = np.ascontiguousarray(np.asarray(kern, np.float32)[::-1, ::-1])
    bands = _build_bands(kern).astype(bf16)
    wfb = np.broadcast_to(wf.reshape(1, 16), (128, 16)).copy().astype(np.float32)
    nc = _build_nc()
    # [C,H,W] -> padded [H,C,262] bf16 (2 zero | 256 data | 4 zero) so every
    # device DMA line is one 4KB-contiguous run (128 descriptors per tile)
    in_maps = []
    for b in range(_NCORES):
        xp = np.zeros((_H, _C, _NW), bf16)
        xp[:, :, 2:258] = x[b].transpose(1, 0, 2).astype(bf16)
        in_maps.append({"x": xp, "bands": bands, "wfb": wfb})
    res = run_bass_kernel_spmd(nc, in_maps, list(range(_NCORES)), trace=trace)
    out = np.stack(
        [
            np.asarray(res.results[i]["out"])[:, :, :_WO]
            .transpose(1, 0, 2)
            .astype(np.float32)
            for i in range(_NCORES)
        ],
        axis=0,
    )
    return out, res


def kernel(x, kernel):
    out, _ = _run(x, kernel, trace=False)
    return out
